# revision 1
# baseline (speedup 1.0000x reference)
"""Trainium2 Bass kernel for nn_HCF_module (SC2 NMS/registration pipeline).

Sharding: 512 seeds split across 8 NeuronCores (64 seeds/core, keypoints
replicated). Device launches (SPMD on cores 0-7 via run_bass_kernel_spmd):
  L1: per-seed top-200 extraction over SC2 rows (exact jax top_k tie order
      via DVE max/max_index/match_replace rounds)
  L2-L5: filter stages k=200/100/50/25 -> per-seed SC2 consistency scores
      (elementwise pairwise-d2 + sqrt-free hard-bit test + row-0 product)
  L6: fitness counts (rigid-transform inlier counting over all 2048 pts)
Host glue between launches: index gathers, final k=12 Kabsch (f32).
"""
import numpy as np

F32 = np.float32
T2 = F32(0.1) * F32(0.1)            # 0.010000000707...
TWO_T2 = F32(2.0) * T2
T4 = T2 * T2
NCORES = 8
SEEDS = 512
SPC = SEEDS // NCORES               # seeds per core
NPTS = 2048

_programs = {}
_launch_wall = []


def _mk_bass():
    import concourse.bass as bass
    return bass.Bass("TRN2", target_bir_lowering=False)


def _prog_topk():
    """[128, 1024] f32 (row 2s+h = seed s, half h) -> top-136 values+idx per half.
    Outputs ym [128,136] f32, yi [128,136] uint32 (local idx in half)."""
    import concourse.mybir as mybir
    nc = _mk_bass()
    P, HN, R = 128, NPTS // 2, 17
    x = nc.dram_tensor("x", [P, HN], mybir.dt.float32, kind="ExternalInput")
    ym = nc.dram_tensor("ym", [P, 8 * R], mybir.dt.float32, kind="ExternalOutput")
    yi = nc.dram_tensor("yi", [P, 8 * R], mybir.dt.uint32, kind="ExternalOutput")
    ctx = nc.ctx
    t = ctx.enter_context(nc.sbuf_tensor([P, HN], mybir.dt.float32))
    m8 = ctx.enter_context(nc.sbuf_tensor([P, 8 * R], mybir.dt.float32))
    i8 = ctx.enter_context(nc.sbuf_tensor([P, 8 * R], mybir.dt.uint32))
    dma_sem = ctx.enter_context(nc.semaphore())
    vsem = ctx.enter_context(nc.semaphore())
    with nc.Block() as block:
        @block.gpsimd
        def _(gpsimd):
            gpsimd.dma_start(t[:, :], x[:, :]).then_inc(dma_sem, 16)
            gpsimd.wait_ge(vsem, 3 * R)
            gpsimd.dma_start(ym[:, :], m8[:, :]).then_inc(dma_sem, 16)
            gpsimd.dma_start(yi[:, :], i8[:, :]).then_inc(dma_sem, 16)
            gpsimd.wait_ge(dma_sem, 48)

        @block.vector
        def _(vector):
            vector.wait_ge(dma_sem, 16)
            n = 0
            for r in range(R):
                sl = slice(r * 8, (r + 1) * 8)
                nc.vector.max(out=m8[:, sl], in_=t[:, :]).then_inc(vsem, 1)
                n += 1
                vector.wait_ge(vsem, n)
                nc.vector.max_index(out=i8[:, sl], in_max=m8[:, sl],
                                    in_values=t[:, :]).then_inc(vsem, 1)
                n += 1
                nc.vector.match_replace(out=t[:, :], in_to_replace=m8[:, sl],
                                        in_values=t[:, :], imm_value=-1e30).then_inc(vsem, 1)
                n += 1
                vector.wait_ge(vsem, n)
    return nc


def _prog_sc2(k):
    """gx,gy [SPC, 3*k] f32 (c-major: x|y|z rows) -> sc2 [SPC, k] f32."""
    import concourse.mybir as mybir
    from concourse.alu_op_type import AluOpType as OP
    nc = _mk_bass()
    gx = nc.dram_tensor("gx", [SPC, 3 * k], mybir.dt.float32, kind="ExternalInput")
    gy = nc.dram_tensor("gy", [SPC, 3 * k], mybir.dt.float32, kind="ExternalInput")
    out = nc.dram_tensor("sc2", [SPC, k], mybir.dt.float32, kind="ExternalOutput")
    ctx = nc.ctx
    B = 20 if k % 20 == 0 else 25  # k=200/100 -> 20, k=50/25 -> 25
    if k % B:
        B = 5
    assert k % B == 0
    tx = ctx.enter_context(nc.sbuf_tensor([SPC, 3 * k], mybir.dt.float32))
    ty = ctx.enter_context(nc.sbuf_tensor([SPC, 3 * k], mybir.dt.float32))
    dxs = ctx.enter_context(nc.sbuf_tensor([SPC, B * 3 * k], mybir.dt.float32))
    d2a = ctx.enter_context(nc.sbuf_tensor([SPC, B * k], mybir.dt.float32))
    d2b = ctx.enter_context(nc.sbuf_tensor([SPC, B * k], mybir.dt.float32))
    q = ctx.enter_context(nc.sbuf_tensor([SPC, B * k], mybir.dt.float32))
    p = ctx.enter_context(nc.sbuf_tensor([SPC, B * k], mybir.dt.float32))
    hard = ctx.enter_context(nc.sbuf_tensor([SPC, B * k], mybir.dt.float32))
    scr = ctx.enter_context(nc.sbuf_tensor([SPC, B * k], mybir.dt.float32))
    h0 = ctx.enter_context(nc.sbuf_tensor([SPC, k], mybir.dt.float32))
    sc2 = ctx.enter_context(nc.sbuf_tensor([SPC, k], mybir.dt.float32))
    dma_sem = ctx.enter_context(nc.semaphore())
    vsem = ctx.enter_context(nc.semaphore())
    nb = k // B
    vcount = [0]

    veng = [None]

    def _fence():
        veng[0].wait_ge(vsem, vcount[0])

    def tt(out_ap, a_ap, b_ap, op):
        nc.vector.tensor_tensor(out=out_ap, in0=a_ap, in1=b_ap, op=op).then_inc(vsem, 1)
        vcount[0] += 1
        _fence()

    def ts(out_ap, a_ap, s1, op0, s2=None, op1=None):
        if op1 is None:
            nc.vector.tensor_scalar(out_ap, a_ap, s1, None, op0).then_inc(vsem, 1)
        else:
            nc.vector.tensor_scalar(out_ap, a_ap, s1, s2, op0, op1).then_inc(vsem, 1)
        vcount[0] += 1
        _fence()

    with nc.Block() as block:
        @block.vector
        def _(vector):
            veng[0] = vector
            vector.wait_ge(dma_sem, 32)
            for bi in range(nb):
                a0 = bi * B
                for (src_t, dst) in ((tx, d2a), (ty, d2b)):
                    v3 = src_t[:, :].rearrange("p (c b) -> p c b", c=3)      # [p,3,k]
                    rows4 = v3.unsqueeze(1).to_broadcast([SPC, B, 3, k])
                    cols4 = v3[:, :, a0:a0 + B].transpose([0, 2, 1]).unsqueeze(3).to_broadcast([SPC, B, 3, k])
                    dx4 = dxs[:, :].rearrange("p (a c b) -> p a c b", a=B, c=3)
                    tt(dx4, rows4, cols4, OP.subtract)
                    tt(dxs[:, :], dxs[:, :], dxs[:, :], OP.mult)
                    d2v = dst[:, :].rearrange("p (a b) -> p a b", a=B)
                    tt(d2v, dx4[:, :, 0, :], dx4[:, :, 1, :], OP.add)
                    tt(d2v, d2v, dx4[:, :, 2, :], OP.add)
                tt(q[:, :], d2a[:, :], d2b[:, :], OP.add)
                tt(p[:, :], d2a[:, :], d2b[:, :], OP.subtract)
                tt(p[:, :], p[:, :], p[:, :], OP.mult)
                ts(scr[:, :], q[:, :], float(TWO_T2), OP.mult, float(T4), OP.subtract)
                tt(hard[:, :], p[:, :], scr[:, :], OP.is_lt)
                ts(scr[:, :], q[:, :], float(T2), OP.is_lt)
                tt(hard[:, :], hard[:, :], scr[:, :], OP.max)
                if bi == 0:
                    nc.vector.tensor_copy(h0[:, :], hard[:, :k]).then_inc(vsem, 1)
                    vcount[0] += 1
                    _fence()
                hv = hard[:, :].rearrange("p (a b) -> p a b", a=B)
                h0c = h0[:, a0:a0 + B].unsqueeze(2).to_broadcast([SPC, B, k])
                tt(hv, hv, h0c, OP.mult)
                hT = hv.transpose([0, 2, 1])                                  # [p,k,a]
                if bi == 0:
                    nc.vector.tensor_reduce(out=sc2[:, :], in_=hT, axis=mybir.AxisListType.X,
                                            op=OP.add).then_inc(vsem, 1)
                    vcount[0] += 1
                    _fence()
                else:
                    nc.vector.tensor_reduce(out=scr[:, :k], in_=hT, axis=mybir.AxisListType.X,
                                            op=OP.add).then_inc(vsem, 1)
                    vcount[0] += 1
                    _fence()
                    tt(sc2[:, :], sc2[:, :], scr[:, :k], OP.add)

        @block.gpsimd
        def _(gpsimd):
            gpsimd.dma_start(tx[:, :], gx[:, :]).then_inc(dma_sem, 16)
            gpsimd.dma_start(ty[:, :], gy[:, :]).then_inc(dma_sem, 16)
            gpsimd.wait_ge(vsem, vcount[0])
            gpsimd.dma_start(out[:, :], sc2[:, :]).then_inc(dma_sem, 16)
            gpsimd.wait_ge(dma_sem, 48)
    return nc


def _prog_fitness():
    """srcb,tgtb [128, 3*1024] (c-major halves), r12 [128, 12] -> cnt [128, 1]."""
    import concourse.mybir as mybir
    from concourse.alu_op_type import AluOpType as OP
    nc = _mk_bass()
    P, HN = 128, NPTS // 2
    srcb = nc.dram_tensor("srcb", [P, 3 * HN], mybir.dt.float32, kind="ExternalInput")
    tgtb = nc.dram_tensor("tgtb", [P, 3 * HN], mybir.dt.float32, kind="ExternalInput")
    r12 = nc.dram_tensor("r12", [P, 12], mybir.dt.float32, kind="ExternalInput")
    cnt = nc.dram_tensor("cnt", [P, 1], mybir.dt.float32, kind="ExternalOutput")
    ctx = nc.ctx
    ts_ = ctx.enter_context(nc.sbuf_tensor([P, 3 * HN], mybir.dt.float32))
    tt_ = ctx.enter_context(nc.sbuf_tensor([P, 3 * HN], mybir.dt.float32))
    tr = ctx.enter_context(nc.sbuf_tensor([P, 12], mybir.dt.float32))
    acc = ctx.enter_context(nc.sbuf_tensor([P, HN], mybir.dt.float32))
    dc = ctx.enter_context(nc.sbuf_tensor([P, 3 * HN], mybir.dt.float32))
    l2s = ctx.enter_context(nc.sbuf_tensor([P, HN], mybir.dt.float32))
    sq = ctx.enter_context(nc.sbuf_tensor([P, HN], mybir.dt.float32))
    ccol = ctx.enter_context(nc.sbuf_tensor([P, 1], mybir.dt.float32))
    dma_sem = ctx.enter_context(nc.semaphore())
    vsem = ctx.enter_context(nc.semaphore())
    vcount = [0]

    with nc.Block() as block:
        @block.vector
        def _(vector):
            def fence():
                vector.wait_ge(vsem, vcount[0])

            def emit(inst):
                inst.then_inc(vsem, 1)
                vcount[0] += 1
                fence()

            vector.wait_ge(dma_sem, 48)
            xv = ts_[:, :].rearrange("p (c b) -> p c b", c=3)
            yvv = tt_[:, :].rearrange("p (c b) -> p c b", c=3)
            dv = dc[:, :].rearrange("p (c b) -> p c b", c=3)
            for c in range(3):
                emit(nc.vector.tensor_scalar(acc[:, :], xv[:, 0, :], tr[:, 4 * c:4 * c + 1],
                                             tr[:, 4 * c + 3:4 * c + 4], OP.mult, OP.add))
                for j in (1, 2):
                    emit(nc.vector.scalar_tensor_tensor(
                        out=acc[:, :], in0=xv[:, j, :], scalar=tr[:, 4 * c + j:4 * c + j + 1],
                        in1=acc[:, :], op0=OP.mult, op1=OP.add))
                emit(nc.vector.tensor_tensor(out=dv[:, c, :], in0=acc[:, :], in1=yvv[:, c, :],
                                             op=OP.subtract))
            emit(nc.vector.tensor_tensor(out=l2s[:, :], in0=dv[:, 0, :], in1=dv[:, 0, :], op=OP.mult))
            emit(nc.vector.tensor_tensor(out=sq[:, :], in0=dv[:, 1, :], in1=dv[:, 1, :], op=OP.mult))
            emit(nc.vector.tensor_tensor(out=l2s[:, :], in0=l2s[:, :], in1=sq[:, :], op=OP.add))
            emit(nc.vector.tensor_tensor(out=sq[:, :], in0=dv[:, 2, :], in1=dv[:, 2, :], op=OP.mult))
            emit(nc.vector.tensor_tensor(out=l2s[:, :], in0=l2s[:, :], in1=sq[:, :], op=OP.add))
            emit(nc.vector.tensor_scalar(sq[:, :], l2s[:, :], float(T2), None, OP.is_lt))
            emit(nc.vector.tensor_reduce(out=ccol[:, :], in_=sq[:, :], axis=mybir.AxisListType.X,
                                         op=OP.add))

        @block.gpsimd
        def _(gpsimd):
            gpsimd.dma_start(ts_[:, :], srcb[:, :]).then_inc(dma_sem, 16)
            gpsimd.dma_start(tt_[:, :], tgtb[:, :]).then_inc(dma_sem, 16)
            gpsimd.dma_start(tr[:, :], r12[:, :]).then_inc(dma_sem, 16)
            gpsimd.wait_ge(vsem, vcount[0])
            gpsimd.dma_start(cnt[:, :], ccol[:, :]).then_inc(dma_sem, 16)
            gpsimd.wait_ge(dma_sem, 64)
    return nc


def _get_prog(key, builder):
    if key not in _programs:
        _programs[key] = builder()
    return _programs[key]


def _run(nc, in_maps):
    import time
    from concourse.bass_utils import run_bass_kernel_spmd
    last = None
    for attempt in range(3):
        try:
            t0 = time.time()
            res = run_bass_kernel_spmd(nc, in_maps, core_ids=list(range(NCORES)))
            _launch_wall.append(time.time() - t0)
            return res.results
        except Exception as e:  # transient device errors: retry
            last = e
    raise last


# ---------------- host-side math (validated f32 device-grade model) -------------

def _topk_host(vals, kk):
    return np.argsort(-vals, axis=-1, kind='stable')[..., :kk]


def _recip(x):
    return (np.float64(1.0) / x.astype(np.float64)).astype(F32)


def _sqrt32(x):
    return np.sqrt(x.astype(np.float64)).astype(F32)


def _cross3(a, b):
    c0 = (a[..., 1] * b[..., 2]).astype(F32) - (a[..., 2] * b[..., 1]).astype(F32)
    c1 = (a[..., 2] * b[..., 0]).astype(F32) - (a[..., 0] * b[..., 2]).astype(F32)
    c2 = (a[..., 0] * b[..., 1]).astype(F32) - (a[..., 1] * b[..., 0]).astype(F32)
    return np.stack([c0.astype(F32), c1.astype(F32), c2.astype(F32)], -1)


def _eig3(K):
    S = K.shape[0]
    qq = ((K[:, 0, 0] + K[:, 1, 1]).astype(F32) + K[:, 2, 2]).astype(F32) * F32(1 / 3)
    qq = qq.astype(F32)
    K00 = (K[:, 0, 0] - qq).astype(F32); K11 = (K[:, 1, 1] - qq).astype(F32); K22 = (K[:, 2, 2] - qq).astype(F32)
    p1 = ((K[:, 0, 1] ** 2).astype(F32) + (K[:, 0, 2] ** 2).astype(F32) + (K[:, 1, 2] ** 2).astype(F32)).astype(F32)
    p2 = ((K00 ** 2).astype(F32) + (K11 ** 2).astype(F32) + (K22 ** 2).astype(F32) + (F32(2) * p1).astype(F32)).astype(F32)
    p = _sqrt32((p2 * F32(1 / 6)).astype(F32))
    rp = _recip(np.maximum(p, F32(1e-30)))
    B00 = (K00 * rp).astype(F32); B11 = (K11 * rp).astype(F32); B22 = (K22 * rp).astype(F32)
    B01 = (K[:, 0, 1] * rp).astype(F32); B02 = (K[:, 0, 2] * rp).astype(F32); B12 = (K[:, 1, 2] * rp).astype(F32)
    detB = (B00 * ((B11 * B22).astype(F32) - (B12 * B12).astype(F32)).astype(F32)).astype(F32) \
        - (B01 * ((B01 * B22).astype(F32) - (B12 * B02).astype(F32)).astype(F32)).astype(F32) \
        + (B02 * ((B01 * B12).astype(F32) - (B11 * B02).astype(F32)).astype(F32)).astype(F32)
    r = np.clip((detB.astype(F32) * F32(0.5)).astype(F32), F32(-1), F32(1))
    c = np.ones(S, F32)
    for _ in range(6):
        f = ((F32(4) * c * c * c).astype(F32) - (F32(3) * c).astype(F32) - r).astype(F32)
        fp = ((F32(12) * c * c).astype(F32) - F32(3)).astype(F32)
        c = np.clip((c - (f * _recip(np.maximum(fp, F32(1e-6)))).astype(F32)).astype(F32), F32(0.5), F32(1.0))
    s_ = _sqrt32(np.maximum((F32(1) - (c * c).astype(F32)).astype(F32), F32(0)))
    lam1 = (qq + (F32(2) * p * c).astype(F32)).astype(F32)
    cmid = ((F32(-0.5) * c).astype(F32) + (F32(np.sqrt(3) / 2) * s_).astype(F32)).astype(F32)
    lam2 = (qq + (F32(2) * p * cmid).astype(F32)).astype(F32)
    return lam1, lam2


def _eigvec(K, lam):
    A = K.astype(F32).copy()
    for i in range(3):
        A[:, i, i] = (A[:, i, i] - lam).astype(F32)
    r0, r1, r2 = A[:, 0, :], A[:, 1, :], A[:, 2, :]
    c1 = _cross3(r0, r1); c2 = _cross3(r1, r2); c3 = _cross3(r2, r0)
    n1 = (c1 ** 2).sum(-1).astype(F32); n2 = (c2 ** 2).sum(-1).astype(F32); n3 = (c3 ** 2).sum(-1).astype(F32)
    a1 = (n1 >= n2) & (n1 >= n3); a2 = (~a1) & (n2 >= n3); a3 = ~(a1 | a2)
    u = (c1 * a1[:, None] + c2 * a2[:, None] + c3 * a3[:, None]).astype(F32)
    n = (u ** 2).sum(-1).astype(F32)
    return (u * _recip(_sqrt32(np.maximum(n, F32(1e-38))))[:, None]).astype(F32)


def _kabsch(A, B, w):
    S = A.shape[0]
    wsum = w.sum(axis=1, dtype=np.float32)
    rws = _recip((wsum + F32(1e-6)).astype(F32))
    wA = (A * w[:, :, None]).astype(F32); wB = (B * w[:, :, None]).astype(F32)
    cA = (wA.sum(axis=1, dtype=np.float32) * rws[:, None]).astype(F32)
    cB = (wB.sum(axis=1, dtype=np.float32) * rws[:, None]).astype(F32)
    Am = (A - cA[:, None, :]).astype(F32); Bm = (B - cB[:, None, :]).astype(F32)
    wAm = (Am * w[:, :, None]).astype(F32)
    H = np.einsum('ski,skj->sij', wAm, Bm).astype(F32)
    K = np.einsum('sij,skj->sik', H, H).astype(F32)
    lam1, lam2 = _eig3(K)
    u1 = _eigvec(K, lam1)
    u2r = _eigvec(K, lam2)
    dot = (u1 * u2r).sum(-1).astype(F32)
    u2 = (u2r - u1 * dot[:, None]).astype(F32)
    n = (u2 ** 2).sum(-1).astype(F32)
    u2 = (u2 * _recip(_sqrt32(np.maximum(n, F32(1e-38))))[:, None]).astype(F32)
    u3 = _cross3(u1, u2)
    w1 = np.einsum('ski,sk->si', H, u1).astype(F32)
    w2 = np.einsum('ski,sk->si', H, u2).astype(F32)
    v1 = (w1 * _recip(_sqrt32(np.maximum((w1 ** 2).sum(-1).astype(F32), F32(1e-38))))[:, None]).astype(F32)
    v2 = (w2 * _recip(_sqrt32(np.maximum((w2 ** 2).sum(-1).astype(F32), F32(1e-38))))[:, None]).astype(F32)
    v3 = _cross3(v1, v2)
    R = (v1[:, :, None] * u1[:, None, :] + v2[:, :, None] * u2[:, None, :]
         + v3[:, :, None] * u3[:, None, :]).astype(F32)
    t = (cB - np.einsum('sij,sj->si', R, cA).astype(F32)).astype(F32)
    return R, t


def _power_iter(M):
    S, k, _ = M.shape
    v = np.ones((S, k), F32)
    for _ in range(10):
        prod = (M * v[:, None, :]).astype(F32)
        acc = prod[:, :, 0]
        for j in range(1, k):
            acc = (acc + prod[:, :, j]).astype(F32)
        n2 = (acc * acc).astype(F32)
        s2 = n2[:, 0]
        for j in range(1, k):
            s2 = (s2 + n2[:, j]).astype(F32)
        nn_ = _sqrt32(s2)
        v = (acc * _recip((nn_ + F32(1e-6)).astype(F32))[:, None]).astype(F32)
    return v


def _pdist2(pts):
    d = (pts[:, :, None, :] - pts[:, None, :, :]).astype(F32)
    sq = (d * d).astype(F32)
    return ((sq[..., 0] + sq[..., 1]).astype(F32) + sq[..., 2]).astype(F32)


def kernel(SC2_measure, src_keypts, tgt_keypts):
    _launch_wall.clear()
    SC2 = np.ascontiguousarray(SC2_measure[0], dtype=np.float32)      # [512, 2048]
    src = np.ascontiguousarray(src_keypts[0], dtype=np.float32)       # [2048, 3]
    tgt = np.ascontiguousarray(tgt_keypts[0], dtype=np.float32)

    # ---- L1: per-seed top-200 on device (rows split into 2 halves) ----
    nc1 = _get_prog("topk", _prog_topk)
    HN = NPTS // 2
    xh = SC2.reshape(SEEDS, 2, HN).reshape(SEEDS * 2, HN)  # row 2s+h
    in_maps = [{"x": xh[c * 2 * SPC:(c + 1) * 2 * SPC]} for c in range(NCORES)]
    for _try in range(4):
        res = _run(nc1, in_maps)
        vm = np.concatenate([res[c]["ym"] for c in range(NCORES)], axis=0)
        vi = np.concatenate([res[c]["yi"] for c in range(NCORES)], axis=0).astype(np.int64)
        if (vi < HN).all():
            break
    # merge halves: concat [A|B]; stable sort by value desc == jax global order
    NE = vm.shape[1]
    cand_v = np.concatenate([vm[0::2], vm[1::2]], axis=1)            # [512, 2*NE]
    cand_i = np.concatenate([vi[0::2], vi[1::2] + HN], axis=1)
    order = np.argsort(-cand_v, axis=1, kind='stable')[:, :200]
    knn = np.take_along_axis(cand_i, order, axis=1)                  # [512, 200]
    # safety: if any seed's 200th value ties the last extracted value of a
    # half, extraction may be incomplete -> exact host fallback for that seed
    thr = np.take_along_axis(cand_v, order[:, 199:200], axis=1)[:, 0]
    risky = (vm[0::2, NE - 1] >= thr) | (vm[1::2, NE - 1] >= thr)
    for s in np.where(risky)[0]:
        knn[s] = np.argsort(-SC2[s], kind='stable')[:200]
    sknn = src[knn].astype(F32)                                       # [512, 200, 3]
    tknn = tgt[knn].astype(F32)

    # ---- L2-L5: filter stages on device ----
    k = 200
    while k > 15:
        nck = _get_prog(("sc2", k), lambda kk=k: _prog_sc2(kk))
        gxa = np.ascontiguousarray(np.transpose(sknn, (0, 2, 1)).reshape(SEEDS, 3 * k))
        gya = np.ascontiguousarray(np.transpose(tknn, (0, 2, 1)).reshape(SEEDS, 3 * k))
        in_maps = [{"gx": gxa[c * SPC:(c + 1) * SPC], "gy": gya[c * SPC:(c + 1) * SPC]}
                   for c in range(NCORES)]
        for _try in range(4):
            res = _run(nck, in_maps)
            sc2 = np.concatenate([res[c]["sc2"] for c in range(NCORES)], axis=0)
            ok = (sc2 == np.round(sc2)).all() and (sc2 >= 0).all() and (sc2 <= k).all() and (sc2[:, 0] >= 1).all()
            if ok:
                break
        kf = k // 2
        sel = _topk_host(sc2, kf)                                     # ties: pos asc
        sknn = np.take_along_axis(sknn, sel[:, :, None], axis=1)
        tknn = np.take_along_axis(tknn, sel[:, :, None], axis=1)
        k = kf
    # k == 12

    # ---- host: local_sc, power iteration, Kabsch (validated f32 model) ----
    a2 = _pdist2(sknn); b2 = _pdist2(tknn)
    da = _sqrt32(np.maximum(a2, F32(1e-12)))
    db = _sqrt32(np.maximum(b2, F32(1e-12)))
    cross = np.abs((da - db).astype(F32)).astype(F32)
    local_sc = np.maximum(F32(1.0) - ((cross * cross).astype(F32) / T2).astype(F32), F32(0.0)).astype(F32)
    eye = np.eye(12, dtype=F32)
    M = (local_sc * (F32(1.0) - eye)[None]).astype(F32)
    v = _power_iter(M)
    wsum = v[:, 0].copy()
    for j in range(1, 12):
        wsum = (wsum + v[:, j]).astype(F32)
    w = (v / (wsum[:, None] + F32(1e-6))).astype(F32)
    R, t = _kabsch(sknn, tknn, w)

    # ---- L6: fitness on device ----
    nc6 = _get_prog("fit", _prog_fitness)
    HN = NPTS // 2
    srcb = np.empty((128, 3 * HN), F32); tgtb = np.empty((128, 3 * HN), F32)
    for h in range(2):
        blk = np.transpose(src[h * HN:(h + 1) * HN], (1, 0)).reshape(3 * HN)
        srcb[h::2, :] = blk[None, :]
        blkt = np.transpose(tgt[h * HN:(h + 1) * HN], (1, 0)).reshape(3 * HN)
        tgtb[h::2, :] = blkt[None, :]
    in_maps = []
    for c in range(NCORES):
        r12 = np.zeros((128, 12), F32)
        for s in range(SPC):
            seed = c * SPC + s
            row = np.concatenate([
                [R[seed, 0, 0], R[seed, 0, 1], R[seed, 0, 2], t[seed, 0]],
                [R[seed, 1, 0], R[seed, 1, 1], R[seed, 1, 2], t[seed, 1]],
                [R[seed, 2, 0], R[seed, 2, 1], R[seed, 2, 2], t[seed, 2]]]).astype(F32)
            r12[2 * s, :] = row
            r12[2 * s + 1, :] = row
        in_maps.append({"srcb": srcb, "tgtb": tgtb, "r12": r12})
    for _try in range(4):
        res = _run(nc6, in_maps)
        _cnts = np.concatenate([res[c]["cnt"][:, 0] for c in range(NCORES)])
        if (_cnts == np.round(_cnts)).all() and (_cnts >= 0).all() and (_cnts <= NPTS).all():
            break
    fitness = np.zeros(SEEDS, np.int64)
    for c in range(NCORES):
        cc = res[c]["cnt"][:, 0]
        for s in range(SPC):
            fitness[c * SPC + s] = int(cc[2 * s]) + int(cc[2 * s + 1])

    import os
    if os.environ.get("KDBG"):
        np.save('/tmp/dbg_fit.npy', fitness)
        np.save('/tmp/dbg_R.npy', R); np.save('/tmp/dbg_t.npy', t)
        np.save('/tmp/dbg_sknn.npy', sknn); np.save('/tmp/dbg_knn.npy', knn)
    best = int(np.argmax(fitness))
    T = np.zeros((1, 4, 4), F32)
    T[0, :3, :3] = R[best]
    T[0, :3, 3] = t[best]
    T[0, 3, 3] = 1.0
    return T



# revision 2
# speedup vs baseline: 7.7204x; 7.7204x over previous
"""Trainium2 Bass kernel for nn_HCF_module (SC2 NMS/registration pipeline).

Single fused device launch (SPMD, 8 NeuronCores, 64 seeds/core on
partitions 0..63). Entire pipeline on device:
  P1 top-200 per seed (DVE max/max_index/match_replace, exact jax tie order)
  P2 coordinate gather via PE one-hot matmuls (bit-exact f32)
  P3 200x200 hard-bit consistency matrix H (bf16, 0/1 exact)
  P4 four masked filter stages (rank vectors replicate jax stable top_k
     recursively; no compaction, integer-exact scores)
  P5 final 12-subset compaction (arithmetic one-hot), M12, power iteration
  P6 closed-form weighted Kabsch (3x3 eigendecomposition)
  P7 inlier counting over all 2048 points
Host glue: input layout prep, final argmax over 512 per-seed fitness.

Engines are strictly serialized via semaphores (one global order across
DVE/ACT/PE/Pool+DMA) - launch overhead dominates total time, not device
compute, so scheduling simplicity wins.
"""
import math
from contextlib import ExitStack
import numpy as np

F32 = np.float32
T2 = float(F32(0.1) * F32(0.1))
TWO_T2 = float(F32(2.0) * F32(T2))
T4 = float(F32(T2) * F32(T2))
NCORES = 8
SEEDS = 512
SPC = SEEDS // NCORES
NPTS = 2048
K0 = 200

_programs = {}
_launch_wall = []


class _Ser:
    """Strictly-serial cross-engine schedule, emitted as per-engine streams
    with semaphore handshakes (each instruction waits for its global
    predecessor; compute engines self-fence)."""

    def __init__(self, nc):
        self.nc = nc
        self.steps = []

    def v(self, fn):
        self.steps.append(("v", fn))

    def s(self, fn):
        self.steps.append(("s", fn))

    def g(self, fn):
        self.steps.append(("g", fn))

    def p(self, fn):
        self.steps.append(("p", fn))

    def dma(self, out, in_):
        self.steps.append(("d", lambda e, nc=self.nc: nc.gpsimd.dma_start(out=out, in_=in_)))

    def emit(self):
        nc = self.nc
        ctx = nc.ctx
        sems = {k: ctx.enter_context(nc.semaphore(name=f"sem_{k}")) for k in "vsgdp"}
        incs = {"v": 1, "s": 1, "g": 1, "d": 16, "p": 1}
        waits = []
        counts = {k: 0 for k in incs}
        prev = None
        for kind, fn in self.steps:
            waits.append(prev)
            counts[kind] += incs[kind]
            prev = (kind, counts[kind])
        totals = dict(counts)
        steps = self.steps

        def run_stream(eng_obj, kinds):
            n_done = {k: 0 for k in incs}
            for i, (kind, fn) in enumerate(steps):
                n_done[kind] += incs[kind]
                if kind not in kinds:
                    continue
                w = waits[i]
                if w is not None and not (w[0] == kind):
                    eng_obj.wait_ge(sems[w[0]], w[1])
                inst = fn(eng_obj)
                inst.then_inc(sems[kind], incs[kind])
                if kind != "d":
                    eng_obj.wait_ge(sems[kind], n_done[kind])

        with nc.Block() as block:
            @block.vector
            def _(vector):
                run_stream(vector, ("v",))
                vector.wait_ge(sems["v"], totals["v"])

            @block.scalar
            def _(scalar):
                run_stream(scalar, ("s",))
                if totals["s"]:
                    scalar.wait_ge(sems["s"], totals["s"])

            @block.tensor
            def _(tensor):
                run_stream(tensor, ("p",))
                if totals["p"]:
                    tensor.wait_ge(sems["p"], totals["p"])

            @block.gpsimd
            def _(gpsimd):
                run_stream(gpsimd, ("g", "d"))
                gpsimd.wait_ge(sems["d"], totals["d"])
                if totals["g"]:
                    gpsimd.wait_ge(sems["g"], totals["g"])


def _build():
    import concourse.bass as bass
    import concourse.mybir as mybir
    from concourse.alu_op_type import AluOpType as OP

    AF = mybir.ActivationFunctionType
    DT = mybir.dt
    AX = mybir.AxisListType

    nc = bass.Bass("TRN2", target_bir_lowering=False)
    ctx = nc.ctx

    sc2m = nc.dram_tensor("sc2m", [SPC, NPTS], DT.float32, kind="ExternalInput")
    tchunks = nc.dram_tensor("tchunks", [128, 96], DT.float32, kind="ExternalInput")
    cloudS = nc.dram_tensor("cloudS", [1, 3 * NPTS], DT.float32, kind="ExternalInput")
    cloudT = nc.dram_tensor("cloudT", [1, 3 * NPTS], DT.float32, kind="ExternalInput")
    outT = nc.dram_tensor("outT", [SPC, 16], DT.float32, kind="ExternalOutput")

    def sb(name, shape, dt=DT.float32):
        return ctx.enter_context(nc.sbuf_tensor(name, shape, dt))

    def sbR(es, name, shape, dt=DT.float32):
        return es.enter_context(nc.sbuf_tensor(name, shape, dt, side="right"))

    S = _Ser(nc)
    TT = lambda out, a, b, op: S.v(lambda e: nc.vector.tensor_tensor(out=out, in0=a, in1=b, op=op))
    TS = lambda out, a, s1, s2, op0, op1=None: S.v(
        lambda e: nc.vector.tensor_scalar(out, a, s1, s2, op0)
        if op1 is None else nc.vector.tensor_scalar(out, a, s1, s2, op0, op1))
    CP = lambda out, a: S.v(lambda e: nc.vector.tensor_copy(out, a))
    RD = lambda out, a: S.v(lambda e: nc.vector.tensor_reduce(out=out, in_=a, axis=AX.X, op=OP.add))
    MS = lambda ap, c: S.v(lambda e: nc.vector.memset(ap, c))
    SQRT = lambda out, a: S.s(lambda e: nc.scalar.activation(out=out, in_=a, func=AF.Sqrt))
    RCP = lambda out, a: S.v(lambda e: nc.vector.reciprocal(out=out, in_=a))
    STT = lambda out, a, sc, b, op0, op1: S.v(
        lambda e: nc.vector.scalar_tensor_tensor(out=out, in0=a, scalar=sc, in1=b, op0=op0, op1=op1))

    # ---- P0: loads ----
    es1 = ExitStack()
    xrow = sbR(es1, "xrow", [SPC, NPTS])
    tableS = sb("tableS", [128, 96])
    S.dma(xrow[:, :], sc2m[:, :])
    S.dma(tableS[:, :], tchunks[:, :])

    # ---- P1: top-200 ----
    m8 = sb("m8", [SPC, 8])
    i200 = sb("i200", [SPC, K0], DT.uint32)
    for r in range(K0 // 8):
        sl = slice(8 * r, 8 * r + 8)
        S.v(lambda e, sl=sl: nc.vector.max(out=m8[:, :], in_=xrow[:, :]))
        S.v(lambda e, sl=sl: nc.vector.max_index(out=i200[:, sl], in_max=m8[:, :], in_values=xrow[:, :]))
        S.v(lambda e, sl=sl: nc.vector.match_replace(out=xrow[:, :], in_to_replace=m8[:, :],
                                                     in_values=xrow[:, :], imm_value=-1e30))
    idxI = sb("idxI", [SPC, K0], DT.int32)
    loI = sb("loI", [SPC, K0], DT.int32)
    hiI = sb("hiI", [SPC, K0], DT.int32)
    loF = sb("loF", [SPC, K0])
    hiF = sb("hiF", [SPC, K0])
    CP(idxI[:, :], i200[:, :])
    TS(loI[:, :], idxI[:, :], 127, None, OP.bitwise_and)
    TS(hiI[:, :], idxI[:, :], 7, None, OP.logical_shift_right)
    CP(loF[:, :], loI[:, :])
    CP(hiF[:, :], hiI[:, :])
    es1.close()

    # ---- P2: gather via PE one-hot matmuls ----
    ident = sb("ident", [128, 128])
    S.g(lambda e: nc.gpsimd.memset(ident[:, :], 0.0))
    S.g(lambda e: nc.gpsimd.affine_select(out=ident[:, :], in_=ident[:, :],
                                          compare_op=OP.not_equal, fill=1.0,
                                          base=0, pattern=[[-1, 128]], channel_multiplier=1))
    io128I = sb("io128I", [SPC, 128], DT.int32)
    io128F = sb("io128F", [SPC, 128])
    io16I = sb("io16I", [SPC, 16], DT.int32)
    io16F = sb("io16F", [SPC, 16])
    posI = sb("posI", [SPC, K0], DT.int32)
    posF = sb("posF", [SPC, K0])
    S.g(lambda e: nc.gpsimd.iota(io128I[:, :], pattern=[[1, 128]], base=0, channel_multiplier=0))
    S.g(lambda e: nc.gpsimd.iota(io16I[:, :], pattern=[[1, 16]], base=0, channel_multiplier=0))
    S.g(lambda e: nc.gpsimd.iota(posI[:, :], pattern=[[1, K0]], base=0, channel_multiplier=0))
    CP(io128F[:, :], io128I[:, :])
    CP(io16F[:, :], io16I[:, :])
    CP(posF[:, :], posI[:, :])

    g6 = sb("g6", [SPC, K0, 6])
    es2 = ExitStack()
    ohq = sbR(es2, "ohq", [SPC, 4, 128])
    ohT = sbR(es2, "ohT", [128, 4, 64])
    cmp16 = sbR(es2, "cmp16", [SPC, 4, 16])
    msel = sbR(es2, "msel", [SPC, 4, 16, 6])
    psT = ctx.enter_context(nc.psum_tensor("psT", [128, 64], DT.float32))
    psS = ctx.enter_context(nc.psum_tensor("psS", [SPC, 4, 96], DT.float32))
    for q in range(K0 // 4):
        r0 = 4 * q
        TT(ohq[:, :, :], io128F[:, :].unsqueeze(1).to_broadcast([SPC, 4, 128]),
           loF[:, r0:r0 + 4].unsqueeze(2).to_broadcast([SPC, 4, 128]), OP.is_equal)
        for i in range(4):
            S.p(lambda e, i=i: nc.tensor.transpose(out=psT[:, :], in_=ohq[:, i, :],
                                                   identity=ident[0:SPC, 0:SPC]))
            CP(ohT[:, i, :], psT[:, :])
            S.p(lambda e, i=i: nc.tensor.matmul(out=psS[:, i, :], lhsT=ohT[:, i, :],
                                                rhs=tableS[:, :], start=True, stop=True))
        TT(cmp16[:, :, :], io16F[:, :].unsqueeze(1).to_broadcast([SPC, 4, 16]),
           hiF[:, r0:r0 + 4].unsqueeze(2).to_broadcast([SPC, 4, 16]), OP.is_equal)
        TT(msel[:, :, :, :], psS[:, :, :].rearrange("p a (c x) -> p a c x", c=16),
           cmp16[:, :, :].unsqueeze(3).to_broadcast([SPC, 4, 16, 6]), OP.mult)
        RD(g6[:, r0:r0 + 4, :], msel[:, :, :, :].transpose([0, 1, 3, 2]))
    gx = sb("gx", [SPC, 3, K0])
    gy = sb("gy", [SPC, 3, K0])
    for c in range(3):
        CP(gx[:, c, :], g6[:, :, c])
        CP(gy[:, c, :], g6[:, :, c + 3])
    es2.close()

    # ---- P3: H bits (bf16 200x200) ----
    H = sb("H", [SPC, K0, K0], DT.bfloat16)
    B = 10
    es3 = ExitStack()
    dxs = sbR(es3, "dxs", [SPC, B, 3, K0])
    d2a = sbR(es3, "d2a", [SPC, B, K0])
    d2b = sbR(es3, "d2b", [SPC, B, K0])
    qq = sbR(es3, "qq", [SPC, B, K0])
    for bi in range(K0 // B):
        a0 = bi * B
        for (gsrc, dst) in ((gx, d2a), (gy, d2b)):
            rows4 = gsrc[:, :, :].unsqueeze(1).to_broadcast([SPC, B, 3, K0])
            cols4 = gsrc[:, :, a0:a0 + B].transpose([0, 2, 1]).unsqueeze(3).to_broadcast([SPC, B, 3, K0])
            TT(dxs[:, :, :, :], rows4, cols4, OP.subtract)
            TT(dxs[:, :, :, :], dxs[:, :, :, :], dxs[:, :, :, :], OP.mult)
            TT(dst[:, :, :], dxs[:, :, 0, :], dxs[:, :, 1, :], OP.add)
            TT(dst[:, :, :], dst[:, :, :], dxs[:, :, 2, :], OP.add)
        TT(qq[:, :, :], d2a[:, :, :], d2b[:, :, :], OP.add)
        TT(d2a[:, :, :], d2a[:, :, :], d2b[:, :, :], OP.subtract)
        TT(d2a[:, :, :], d2a[:, :, :], d2a[:, :, :], OP.mult)
        TS(d2b[:, :, :], qq[:, :, :], TWO_T2, T4, OP.mult, OP.subtract)
        TT(d2a[:, :, :], d2a[:, :, :], d2b[:, :, :], OP.is_lt)
        TS(d2b[:, :, :], qq[:, :, :], T2, None, OP.is_lt)
        TT(H[:, a0:a0 + B, :], d2a[:, :, :], d2b[:, :, :], OP.max)
    es3.close()

    # ---- P4: masked filter stages ----
    es4 = ExitStack()
    TMP = sbR(es4, "TMP", [SPC, K0, K0], DT.bfloat16)
    mM = sb("mM", [SPC, K0])
    rF = sb("rF", [SPC, K0])
    lam = sb("lam", [SPC, K0])
    Hl = sb("Hl", [SPC, K0])
    vv = sb("vv", [SPC, K0])
    sc2v = sb("sc2v", [SPC, K0])
    packed = sb("packed", [SPC, K0])
    pcopy = sb("pcopy", [SPC, K0])
    m8s = sb("m8s", [SPC, 104])
    MS(mM[:, :], 1.0)
    CP(rF[:, :], posF[:, :])
    for kf in (100, 50, 25, 12):
        TS(lam[:, :], rF[:, :], 0.0, None, OP.is_equal)
        TT(TMP[:, :, :], H[:, :, :], lam[:, :].unsqueeze(2).to_broadcast([SPC, K0, K0]), OP.mult)
        RD(Hl[:, :], TMP[:, :, :].transpose([0, 2, 1]))
        TT(vv[:, :], Hl[:, :], mM[:, :], OP.mult)
        TT(TMP[:, :, :], H[:, :, :], vv[:, :].unsqueeze(2).to_broadcast([SPC, K0, K0]), OP.mult)
        RD(sc2v[:, :], TMP[:, :, :].transpose([0, 2, 1]))
        TS(packed[:, :], sc2v[:, :], 256.0, 255.0, OP.mult, OP.add)
        TT(packed[:, :], packed[:, :], rF[:, :], OP.subtract)
        TT(packed[:, :], packed[:, :], mM[:, :], OP.mult)
        CP(pcopy[:, :], packed[:, :])
        for r in range(math.ceil(kf / 8)):
            sl = slice(8 * r, 8 * r + 8)
            S.v(lambda e, sl=sl: nc.vector.max(out=m8s[:, sl], in_=pcopy[:, :]))
            S.v(lambda e, sl=sl: nc.vector.match_replace(out=pcopy[:, :], in_to_replace=m8s[:, sl],
                                                         in_values=pcopy[:, :], imm_value=-1.0))
        TS(mM[:, :], packed[:, :], m8s[:, kf - 1:kf], None, OP.is_ge)
        TT(TMP[:, :, :], packed[:, :].unsqueeze(2).to_broadcast([SPC, K0, K0]),
           packed[:, :].unsqueeze(1).to_broadcast([SPC, K0, K0]), OP.is_gt)
        RD(rF[:, :], TMP[:, :, :].transpose([0, 2, 1]))
    es4.close()

    # ---- P5: final compaction + M12 + power iteration ----
    fx = sb("fx", [SPC, 3, 12])
    fy = sb("fy", [SPC, 3, 12])
    oh1 = sb("oh1", [SPC, K0])
    t200 = sb("t200", [SPC, K0])
    for rho in range(12):
        TS(oh1[:, :], rF[:, :], float(rho), None, OP.is_equal)
        for c in range(3):
            TT(t200[:, :], oh1[:, :], gx[:, c, :], OP.mult)
            RD(fx[:, c, rho:rho + 1], t200[:, :])
            TT(t200[:, :], oh1[:, :], gy[:, c, :], OP.mult)
            RD(fy[:, c, rho:rho + 1], t200[:, :])

    dx12 = sb("dx12", [SPC, 12, 3, 12])
    a2s = sb("a2s", [SPC, 12, 12])
    b2s = sb("b2s", [SPC, 12, 12])
    M12 = sb("M12", [SPC, 12, 12])
    for (gsrc, dst) in ((fx, a2s), (fy, b2s)):
        rows4 = gsrc[:, :, :].unsqueeze(1).to_broadcast([SPC, 12, 3, 12])
        cols4 = gsrc[:, :, :].transpose([0, 2, 1]).unsqueeze(3).to_broadcast([SPC, 12, 3, 12])
        TT(dx12[:, :, :, :], rows4, cols4, OP.subtract)
        TT(dx12[:, :, :, :], dx12[:, :, :, :], dx12[:, :, :, :], OP.mult)
        TT(dst[:, :, :], dx12[:, :, 0, :], dx12[:, :, 1, :], OP.add)
        TT(dst[:, :, :], dst[:, :, :], dx12[:, :, 2, :], OP.add)
    TS(a2s[:, :, :], a2s[:, :, :], 1e-12, None, OP.max)
    TS(b2s[:, :, :], b2s[:, :, :], 1e-12, None, OP.max)
    SQRT(a2s[:, :, :], a2s[:, :, :])
    SQRT(b2s[:, :, :], b2s[:, :, :])
    TT(a2s[:, :, :], a2s[:, :, :], b2s[:, :, :], OP.subtract)
    TT(a2s[:, :, :], a2s[:, :, :], a2s[:, :, :], OP.mult)
    TS(M12[:, :, :], a2s[:, :, :], float(F32(1.0) / F32(T2)), None, OP.mult)
    TS(M12[:, :, :], M12[:, :, :], -1.0, None, OP.mult)
    TS(M12[:, :, :], M12[:, :, :], 1.0, None, OP.add)
    TS(M12[:, :, :], M12[:, :, :], 0.0, None, OP.max)
    S.g(lambda e: nc.gpsimd.affine_select(out=M12[:, :, :], in_=M12[:, :, :],
                                          compare_op=OP.not_equal, fill=0.0,
                                          base=0, pattern=[[-1, 12], [1, 12]],
                                          channel_multiplier=0))
    v12 = sb("v12", [SPC, 12])
    t144 = sb("t144", [SPC, 12, 12])
    sq12 = sb("sq12", [SPC, 12])
    nrm = sb("nrm", [SPC, 1])
    MS(v12[:, :], 1.0)
    for _ in range(10):
        TT(t144[:, :, :], M12[:, :, :], v12[:, :].unsqueeze(1).to_broadcast([SPC, 12, 12]), OP.mult)
        RD(v12[:, :], t144[:, :, :])
        TT(sq12[:, :], v12[:, :], v12[:, :], OP.mult)
        RD(nrm[:, :], sq12[:, :])
        SQRT(nrm[:, :], nrm[:, :])
        TS(nrm[:, :], nrm[:, :], 1e-6, None, OP.add)
        RCP(nrm[:, :], nrm[:, :])
        TS(v12[:, :], v12[:, :], nrm[:, 0:1], None, OP.mult)
    w12 = sb("w12", [SPC, 12])
    RD(nrm[:, :], v12[:, :])
    TS(nrm[:, :], nrm[:, :], 1e-6, None, OP.add)
    RCP(nrm[:, :], nrm[:, :])
    TS(w12[:, :], v12[:, :], nrm[:, 0:1], None, OP.mult)

    # ---- P6: Kabsch ----
    t12a = sb("t12a", [SPC, 12])
    t3a = sb("t3a", [SPC, 3])
    cA = sb("cA", [SPC, 3])
    cB = sb("cB", [SPC, 3])
    ws1 = sb("ws1", [SPC, 1])
    Am = sb("Am", [SPC, 3, 12])
    Bm = sb("Bm", [SPC, 3, 12])
    wAm = sb("wAm", [SPC, 3, 12])
    Hm = sb("Hm", [SPC, 9])
    Km = sb("Km", [SPC, 9])
    RD(ws1[:, :], w12[:, :])
    TS(ws1[:, :], ws1[:, :], 1e-6, None, OP.add)
    RCP(ws1[:, :], ws1[:, :])
    for c in range(3):
        TT(t12a[:, :], fx[:, c, :], w12[:, :], OP.mult)
        RD(cA[:, c:c + 1], t12a[:, :])
        TT(t12a[:, :], fy[:, c, :], w12[:, :], OP.mult)
        RD(cB[:, c:c + 1], t12a[:, :])
    TS(cA[:, :], cA[:, :], ws1[:, 0:1], None, OP.mult)
    TS(cB[:, :], cB[:, :], ws1[:, 0:1], None, OP.mult)
    TT(Am[:, :, :], fx[:, :, :], cA[:, :].unsqueeze(2).to_broadcast([SPC, 3, 12]), OP.subtract)
    TT(Bm[:, :, :], fy[:, :, :], cB[:, :].unsqueeze(2).to_broadcast([SPC, 3, 12]), OP.subtract)
    TT(wAm[:, :, :], Am[:, :, :], w12[:, :].unsqueeze(1).to_broadcast([SPC, 3, 12]), OP.mult)
    for i in range(3):
        for j in range(3):
            TT(t12a[:, :], wAm[:, i, :], Bm[:, j, :], OP.mult)
            RD(Hm[:, 3 * i + j:3 * i + j + 1], t12a[:, :])
    for i in range(3):
        for k in range(3):
            TT(t3a[:, :], Hm[:, 3 * i:3 * i + 3], Hm[:, 3 * k:3 * k + 3], OP.mult)
            RD(Km[:, 3 * i + k:3 * i + k + 1], t3a[:, :])

    s1 = lambda name: sb(name, [SPC, 1])
    eqq = s1("eqq"); ts1 = s1("ts1"); ts2 = s1("ts2")
    p1 = s1("p1"); p2v = s1("p2v"); pv = s1("pv"); rp = s1("rp")
    Bk = sb("Bk", [SPC, 9])
    detB = s1("detB"); rr = s1("rr"); cc = s1("cc"); c2 = s1("c2")
    ff = s1("ff"); fp = s1("fp"); ss = s1("ss"); lam1 = s1("lam1"); lam2 = s1("lam2")

    TT(eqq[:, :], Km[:, 0:1], Km[:, 4:5], OP.add)
    TT(eqq[:, :], eqq[:, :], Km[:, 8:9], OP.add)
    TS(eqq[:, :], eqq[:, :], float(F32(1.0) / F32(3.0)), None, OP.mult)
    CP(Bk[:, :], Km[:, :])
    for d in (0, 4, 8):
        TS(Bk[:, d:d + 1], Bk[:, d:d + 1], eqq[:, 0:1], None, OP.subtract)
    TT(p1[:, :], Km[:, 1:2], Km[:, 1:2], OP.mult)
    TT(ts1[:, :], Km[:, 2:3], Km[:, 2:3], OP.mult)
    TT(p1[:, :], p1[:, :], ts1[:, :], OP.add)
    TT(ts1[:, :], Km[:, 5:6], Km[:, 5:6], OP.mult)
    TT(p1[:, :], p1[:, :], ts1[:, :], OP.add)
    TT(p2v[:, :], Bk[:, 0:1], Bk[:, 0:1], OP.mult)
    TT(ts1[:, :], Bk[:, 4:5], Bk[:, 4:5], OP.mult)
    TT(p2v[:, :], p2v[:, :], ts1[:, :], OP.add)
    TT(ts1[:, :], Bk[:, 8:9], Bk[:, 8:9], OP.mult)
    TT(p2v[:, :], p2v[:, :], ts1[:, :], OP.add)
    TS(ts1[:, :], p1[:, :], 2.0, None, OP.mult)
    TT(p2v[:, :], p2v[:, :], ts1[:, :], OP.add)
    TS(pv[:, :], p2v[:, :], float(F32(1.0) / F32(6.0)), None, OP.mult)
    SQRT(pv[:, :], pv[:, :])
    TS(rp[:, :], pv[:, :], 1e-30, None, OP.max)
    RCP(rp[:, :], rp[:, :])
    TS(Bk[:, :], Bk[:, :], rp[:, 0:1], None, OP.mult)
    TT(ts1[:, :], Bk[:, 4:5], Bk[:, 8:9], OP.mult)
    TT(ts2[:, :], Bk[:, 5:6], Bk[:, 5:6], OP.mult)
    TT(ts1[:, :], ts1[:, :], ts2[:, :], OP.subtract)
    TT(detB[:, :], Bk[:, 0:1], ts1[:, :], OP.mult)
    TT(ts1[:, :], Bk[:, 1:2], Bk[:, 8:9], OP.mult)
    TT(ts2[:, :], Bk[:, 5:6], Bk[:, 2:3], OP.mult)
    TT(ts1[:, :], ts1[:, :], ts2[:, :], OP.subtract)
    TT(ts1[:, :], Bk[:, 1:2], ts1[:, :], OP.mult)
    TT(detB[:, :], detB[:, :], ts1[:, :], OP.subtract)
    TT(ts1[:, :], Bk[:, 1:2], Bk[:, 5:6], OP.mult)
    TT(ts2[:, :], Bk[:, 4:5], Bk[:, 2:3], OP.mult)
    TT(ts1[:, :], ts1[:, :], ts2[:, :], OP.subtract)
    TT(ts1[:, :], Bk[:, 2:3], ts1[:, :], OP.mult)
    TT(detB[:, :], detB[:, :], ts1[:, :], OP.add)
    TS(rr[:, :], detB[:, :], 0.5, None, OP.mult)
    TS(rr[:, :], rr[:, :], -1.0, None, OP.max)
    TS(rr[:, :], rr[:, :], 1.0, None, OP.min)
    MS(cc[:, :], 1.0)
    for _ in range(6):
        TT(c2[:, :], cc[:, :], cc[:, :], OP.mult)
        TT(ff[:, :], c2[:, :], cc[:, :], OP.mult)
        TS(ff[:, :], ff[:, :], 4.0, None, OP.mult)
        TS(ts1[:, :], cc[:, :], 3.0, None, OP.mult)
        TT(ff[:, :], ff[:, :], ts1[:, :], OP.subtract)
        TT(ff[:, :], ff[:, :], rr[:, :], OP.subtract)
        TS(fp[:, :], c2[:, :], 12.0, 3.0, OP.mult, OP.subtract)
        TS(fp[:, :], fp[:, :], 1e-6, None, OP.max)
        RCP(fp[:, :], fp[:, :])
        TT(ff[:, :], ff[:, :], fp[:, :], OP.mult)
        TT(cc[:, :], cc[:, :], ff[:, :], OP.subtract)
        TS(cc[:, :], cc[:, :], 0.5, None, OP.max)
        TS(cc[:, :], cc[:, :], 1.0, None, OP.min)
    TT(c2[:, :], cc[:, :], cc[:, :], OP.mult)
    TS(ss[:, :], c2[:, :], -1.0, 1.0, OP.mult, OP.add)
    TS(ss[:, :], ss[:, :], 0.0, None, OP.max)
    SQRT(ss[:, :], ss[:, :])
    TT(ts1[:, :], pv[:, :], cc[:, :], OP.mult)
    TS(ts1[:, :], ts1[:, :], 2.0, None, OP.mult)
    TT(lam1[:, :], eqq[:, :], ts1[:, :], OP.add)
    TS(ts1[:, :], cc[:, :], -0.5, None, OP.mult)
    TS(ts2[:, :], ss[:, :], float(F32(np.sqrt(3.0) / 2.0)), None, OP.mult)
    TT(ts1[:, :], ts1[:, :], ts2[:, :], OP.add)
    TT(ts1[:, :], pv[:, :], ts1[:, :], OP.mult)
    TS(ts1[:, :], ts1[:, :], 2.0, None, OP.mult)
    TT(lam2[:, :], eqq[:, :], ts1[:, :], OP.add)

    Ae = sb("Ae", [SPC, 9])
    cr1 = sb("cr1", [SPC, 3]); cr2 = sb("cr2", [SPC, 3]); cr3 = sb("cr3", [SPC, 3])
    n1 = s1("n1"); n2 = s1("n2"); n3 = s1("n3")
    aa1 = s1("aa1"); aa2 = s1("aa2"); aa3 = s1("aa3")
    u1 = sb("u1", [SPC, 3]); u2 = sb("u2", [SPC, 3]); u3 = sb("u3", [SPC, 3])

    def cross_rows(out, r0s, r1s):
        for (o, x, y) in ((0, 1, 2), (1, 2, 0), (2, 0, 1)):
            TT(ts1[:, :], r0s[:, x:x + 1], r1s[:, y:y + 1], OP.mult)
            TT(ts2[:, :], r0s[:, y:y + 1], r1s[:, x:x + 1], OP.mult)
            TT(out[:, o:o + 1], ts1[:, :], ts2[:, :], OP.subtract)

    def eigvec(uout, lamv):
        CP(Ae[:, :], Km[:, :])
        for d in (0, 4, 8):
            TS(Ae[:, d:d + 1], Ae[:, d:d + 1], lamv[:, 0:1], None, OP.subtract)
        r0s, r1s, r2s = Ae[:, 0:3], Ae[:, 3:6], Ae[:, 6:9]
        cross_rows(cr1, r0s, r1s)
        cross_rows(cr2, r1s, r2s)
        cross_rows(cr3, r2s, r0s)
        for (nv, crv) in ((n1, cr1), (n2, cr2), (n3, cr3)):
            TT(t3a[:, :], crv[:, :], crv[:, :], OP.mult)
            RD(nv[:, :], t3a[:, :])
        TT(aa1[:, :], n1[:, :], n2[:, :], OP.is_ge)
        TT(ts1[:, :], n1[:, :], n3[:, :], OP.is_ge)
        TT(aa1[:, :], aa1[:, :], ts1[:, :], OP.mult)
        TS(aa2[:, :], aa1[:, :], -1.0, 1.0, OP.mult, OP.add)
        TT(ts1[:, :], n2[:, :], n3[:, :], OP.is_ge)
        TT(aa2[:, :], aa2[:, :], ts1[:, :], OP.mult)
        TS(aa3[:, :], aa1[:, :], -1.0, 1.0, OP.mult, OP.add)
        TT(aa3[:, :], aa3[:, :], aa2[:, :], OP.subtract)
        TS(uout[:, :], cr1[:, :], aa1[:, 0:1], None, OP.mult)
        STT(uout[:, :], cr2[:, :], aa2[:, 0:1], uout[:, :], OP.mult, OP.add)
        STT(uout[:, :], cr3[:, :], aa3[:, 0:1], uout[:, :], OP.mult, OP.add)
        TT(t3a[:, :], uout[:, :], uout[:, :], OP.mult)
        RD(ts1[:, :], t3a[:, :])
        TS(ts1[:, :], ts1[:, :], 1e-38, None, OP.max)
        SQRT(ts1[:, :], ts1[:, :])
        RCP(ts1[:, :], ts1[:, :])
        TS(uout[:, :], uout[:, :], ts1[:, 0:1], None, OP.mult)

    eigvec(u1, lam1)
    eigvec(u2, lam2)
    TT(t3a[:, :], u1[:, :], u2[:, :], OP.mult)
    RD(ts1[:, :], t3a[:, :])
    STT(u2[:, :], u1[:, :], ts1[:, 0:1], u2[:, :], OP.mult, OP.subtract)
    TS(u2[:, :], u2[:, :], -1.0, None, OP.mult)
    TT(t3a[:, :], u2[:, :], u2[:, :], OP.mult)
    RD(ts1[:, :], t3a[:, :])
    TS(ts1[:, :], ts1[:, :], 1e-38, None, OP.max)
    SQRT(ts1[:, :], ts1[:, :])
    RCP(ts1[:, :], ts1[:, :])
    TS(u2[:, :], u2[:, :], ts1[:, 0:1], None, OP.mult)
    cross_rows(u3, u1, u2)
    wv1 = sb("wv1", [SPC, 3]); wv2 = sb("wv2", [SPC, 3])
    for i in range(3):
        TT(t3a[:, :], Hm[:, i::3], u1[:, :], OP.mult)
        RD(wv1[:, i:i + 1], t3a[:, :])
        TT(t3a[:, :], Hm[:, i::3], u2[:, :], OP.mult)
        RD(wv2[:, i:i + 1], t3a[:, :])
    for wv in (wv1, wv2):
        TT(t3a[:, :], wv[:, :], wv[:, :], OP.mult)
        RD(ts1[:, :], t3a[:, :])
        TS(ts1[:, :], ts1[:, :], 1e-38, None, OP.max)
        SQRT(ts1[:, :], ts1[:, :])
        RCP(ts1[:, :], ts1[:, :])
        TS(wv[:, :], wv[:, :], ts1[:, 0:1], None, OP.mult)
    vv3 = sb("vv3", [SPC, 3])
    cross_rows(vv3, wv1, wv2)
    R9 = sb("R9", [SPC, 9])
    for c in range(3):
        TS(R9[:, 3 * c:3 * c + 3], u1[:, :], wv1[:, c:c + 1], None, OP.mult)
        STT(R9[:, 3 * c:3 * c + 3], u2[:, :], wv2[:, c:c + 1], R9[:, 3 * c:3 * c + 3], OP.mult, OP.add)
        STT(R9[:, 3 * c:3 * c + 3], u3[:, :], vv3[:, c:c + 1], R9[:, 3 * c:3 * c + 3], OP.mult, OP.add)
    t3v = sb("t3v", [SPC, 3])
    for c in range(3):
        TT(t3a[:, :], R9[:, 3 * c:3 * c + 3], cA[:, :], OP.mult)
        RD(ts1[:, :], t3a[:, :])
        TT(t3v[:, c:c + 1], cB[:, c:c + 1], ts1[:, :], OP.subtract)

    # ---- P7: fitness ----
    es7 = ExitStack()
    clS = sbR(es7, "clS", [SPC, 3 * NPTS])
    clT = sbR(es7, "clT", [SPC, 3 * NPTS])
    acc = sbR(es7, "acc", [SPC, NPTS])
    dcv = sbR(es7, "dcv", [SPC, 3, NPTS])
    l2s = sbR(es7, "l2s", [SPC, NPTS])
    sqv = sbR(es7, "sqv", [SPC, NPTS])
    S.dma(clS[:, :], cloudS[:, :].to_broadcast([SPC, 3 * NPTS]))
    S.dma(clT[:, :], cloudT[:, :].to_broadcast([SPC, 3 * NPTS]))
    cnt = sb("cnt", [SPC, 1])
    xv = clS[:, :].rearrange("p (c n) -> p c n", c=3)
    yv = clT[:, :].rearrange("p (c n) -> p c n", c=3)
    for c in range(3):
        TS(acc[:, :], xv[:, 0, :], R9[:, 3 * c:3 * c + 1], t3v[:, c:c + 1], OP.mult, OP.add)
        STT(acc[:, :], xv[:, 1, :], R9[:, 3 * c + 1:3 * c + 2], acc[:, :], OP.mult, OP.add)
        STT(acc[:, :], xv[:, 2, :], R9[:, 3 * c + 2:3 * c + 3], acc[:, :], OP.mult, OP.add)
        TT(dcv[:, c, :], acc[:, :], yv[:, c, :], OP.subtract)
    TT(l2s[:, :], dcv[:, 0, :], dcv[:, 0, :], OP.mult)
    TT(sqv[:, :], dcv[:, 1, :], dcv[:, 1, :], OP.mult)
    TT(l2s[:, :], l2s[:, :], sqv[:, :], OP.add)
    TT(sqv[:, :], dcv[:, 2, :], dcv[:, 2, :], OP.mult)
    TT(l2s[:, :], l2s[:, :], sqv[:, :], OP.add)
    TS(sqv[:, :], l2s[:, :], T2, None, OP.is_lt)
    RD(cnt[:, :], sqv[:, :])

    res = sb("res", [SPC, 16])
    MS(res[:, :], 0.0)
    CP(res[:, 0:9], R9[:, :])
    CP(res[:, 9:12], t3v[:, :])
    CP(res[:, 12:13], cnt[:, :])
    S.dma(outT[:, :], res[:, :])
    es7.close()
    S.emit()
    return nc


def _get_prog():
    if "fused" not in _programs:
        _programs["fused"] = _build()
    return _programs["fused"]


def _run(nc, in_maps):
    import time
    from concourse.bass_utils import run_bass_kernel_spmd
    last = None
    for attempt in range(3):
        try:
            t0 = time.time()
            res = run_bass_kernel_spmd(nc, in_maps, core_ids=list(range(NCORES)))
            _launch_wall.append(time.time() - t0)
            return res.results
        except Exception as e:   # transient device errors: retry
            last = e
    raise last


def kernel(SC2_measure, src_keypts, tgt_keypts):
    _launch_wall.clear()
    SC2 = np.ascontiguousarray(SC2_measure[0], dtype=F32)       # [512, 2048]
    src = np.ascontiguousarray(src_keypts[0], dtype=F32)        # [2048, 3]
    tgt = np.ascontiguousarray(tgt_keypts[0], dtype=F32)

    table6 = np.concatenate([src, tgt], axis=1).astype(F32)     # [2048, 6]
    tchunks = np.ascontiguousarray(
        table6.reshape(16, 128, 6).transpose(1, 0, 2).reshape(128, 96))
    cS = np.ascontiguousarray(src.T.reshape(1, 3 * NPTS)).astype(F32)
    cT = np.ascontiguousarray(tgt.T.reshape(1, 3 * NPTS)).astype(F32)
    in_maps = [{
        "sc2m": np.ascontiguousarray(SC2[c * SPC:(c + 1) * SPC]),
        "tchunks": tchunks, "cloudS": cS, "cloudT": cT,
    } for c in range(NCORES)]

    nc = _get_prog()
    for _try in range(3):
        res = _run(nc, in_maps)
        out = np.concatenate([res[c]["outT"] for c in range(NCORES)], axis=0)
        fit = out[:, 12]
        rnorm = (out[:, 0:9] ** 2).sum(axis=1)
        ok = ((fit == np.round(fit)).all() and (fit >= 0).all() and (fit <= NPTS).all()
              and np.isfinite(out).all() and (np.abs(rnorm - 3.0) < 0.5).all())
        if ok:
            break
    best = int(np.argmax(fit))
    T = np.zeros((1, 4, 4), F32)
    T[0, :3, :3] = out[best, 0:9].reshape(3, 3)
    T[0, :3, 3] = out[best, 9:12]
    T[0, 3, 3] = 1.0
    return T


# revision 3
# speedup vs baseline: 16.1047x; 2.0860x over previous
"""Trainium2 Bass kernel for nn_HCF_module (SC2 NMS/registration pipeline).

Single fused device launch (SPMD, 8 NeuronCores, 64 seeds/core on
partitions 0..63). Entire pipeline on device:
  P1 top-200 per seed (DVE max/max_index/match_replace, exact jax tie order)
  P2 coordinate gather via PE one-hot matmuls (bit-exact f32)
  P3 200x200 hard-bit consistency matrix H (bf16, 0/1 exact)
  P4 four masked filter stages (rank vectors replicate jax stable top_k
     recursively; no compaction, integer-exact scores)
  P5 final 12-subset compaction (arithmetic one-hot), M12, power iteration
  P6 closed-form weighted Kabsch (3x3 eigendecomposition)
  P7 inlier counting over all 2048 points
Host glue: input layout prep, final argmax over 512 per-seed fitness.

Engines are strictly serialized via semaphores (one global order across
DVE/ACT/PE/Pool+DMA) - launch overhead dominates total time, not device
compute, so scheduling simplicity wins.
"""
import math
from contextlib import ExitStack
import numpy as np

F32 = np.float32
T2 = float(F32(0.1) * F32(0.1))
TWO_T2 = float(F32(2.0) * F32(T2))
T4 = float(F32(T2) * F32(T2))
NCORES = 8
SEEDS = 512
SPC = SEEDS // NCORES
NPTS = 2048
K0 = 200

_programs = {}
_launch_wall = []


class _Ser:
    """Strictly-serial cross-engine schedule, emitted as per-engine streams
    with semaphore handshakes (each instruction waits for its global
    predecessor; compute engines self-fence)."""

    def __init__(self, nc):
        self.nc = nc
        self.steps = []

    def v(self, fn):
        self.steps.append(("v", fn))

    def s(self, fn):
        self.steps.append(("s", fn))

    def g(self, fn):
        self.steps.append(("g", fn))

    def p(self, fn):
        self.steps.append(("p", fn))

    def dma(self, out, in_):
        self.steps.append(("d", lambda e, nc=self.nc: nc.gpsimd.dma_start(out=out, in_=in_)))

    def emit(self):
        nc = self.nc
        ctx = nc.ctx
        sems = {k: ctx.enter_context(nc.semaphore(name=f"sem_{k}")) for k in "vsgdp"}
        incs = {"v": 1, "s": 1, "g": 1, "d": 16, "p": 1}
        waits = []
        counts = {k: 0 for k in incs}
        prev = None
        for kind, fn in self.steps:
            waits.append(prev)
            counts[kind] += incs[kind]
            prev = (kind, counts[kind])
        totals = dict(counts)
        steps = self.steps

        def run_stream(eng_obj, kinds):
            n_done = {k: 0 for k in incs}
            for i, (kind, fn) in enumerate(steps):
                n_done[kind] += incs[kind]
                if kind not in kinds:
                    continue
                w = waits[i]
                if w is not None and not (w[0] == kind):
                    eng_obj.wait_ge(sems[w[0]], w[1])
                inst = fn(eng_obj)
                inst.then_inc(sems[kind], incs[kind])
                if kind != "d":
                    eng_obj.wait_ge(sems[kind], n_done[kind])

        with nc.Block() as block:
            @block.vector
            def _(vector):
                run_stream(vector, ("v",))
                vector.wait_ge(sems["v"], totals["v"])

            @block.scalar
            def _(scalar):
                run_stream(scalar, ("s",))
                if totals["s"]:
                    scalar.wait_ge(sems["s"], totals["s"])

            @block.tensor
            def _(tensor):
                run_stream(tensor, ("p",))
                if totals["p"]:
                    tensor.wait_ge(sems["p"], totals["p"])

            @block.gpsimd
            def _(gpsimd):
                run_stream(gpsimd, ("g", "d"))
                gpsimd.wait_ge(sems["d"], totals["d"])
                if totals["g"]:
                    gpsimd.wait_ge(sems["g"], totals["g"])


def _build():
    import concourse.bass as bass
    import concourse.mybir as mybir
    from concourse.alu_op_type import AluOpType as OP

    AF = mybir.ActivationFunctionType
    DT = mybir.dt
    AX = mybir.AxisListType

    nc = bass.Bass("TRN2", target_bir_lowering=False)
    ctx = nc.ctx

    sc2m = nc.dram_tensor("sc2m", [SPC, NPTS], DT.float32, kind="ExternalInput")
    tchunks = nc.dram_tensor("tchunks", [128, 96], DT.float32, kind="ExternalInput")
    cloudS = nc.dram_tensor("cloudS", [1, 3 * NPTS], DT.float32, kind="ExternalInput")
    cloudT = nc.dram_tensor("cloudT", [1, 3 * NPTS], DT.float32, kind="ExternalInput")
    outT = nc.dram_tensor("outT", [SPC, 16], DT.float32, kind="ExternalOutput")

    def sb(name, shape, dt=DT.float32):
        return ctx.enter_context(nc.sbuf_tensor(name, shape, dt))

    def sbR(es, name, shape, dt=DT.float32):
        return es.enter_context(nc.sbuf_tensor(name, shape, dt, side="right"))

    S = _Ser(nc)
    TT = lambda out, a, b, op: S.v(lambda e: nc.vector.tensor_tensor(out=out, in0=a, in1=b, op=op))
    TS = lambda out, a, s1, s2, op0, op1=None: S.v(
        lambda e: nc.vector.tensor_scalar(out, a, s1, s2, op0)
        if op1 is None else nc.vector.tensor_scalar(out, a, s1, s2, op0, op1))
    CP = lambda out, a: S.v(lambda e: nc.vector.tensor_copy(out, a))
    RD = lambda out, a: S.v(lambda e: nc.vector.tensor_reduce(out=out, in_=a, axis=AX.X, op=OP.add))
    MS = lambda ap, c: S.v(lambda e: nc.vector.memset(ap, c))
    SQRT = lambda out, a: S.s(lambda e: nc.scalar.activation(out=out, in_=a, func=AF.Sqrt))
    RCP = lambda out, a: S.v(lambda e: nc.vector.reciprocal(out=out, in_=a))
    STT = lambda out, a, sc, b, op0, op1: S.v(
        lambda e: nc.vector.scalar_tensor_tensor(out=out, in0=a, scalar=sc, in1=b, op0=op0, op1=op1))

    # ---- P0: loads ----
    es1 = ExitStack()
    xrow = sbR(es1, "xrow", [SPC, NPTS])
    tableS = sb("tableS", [128, 96])
    S.dma(xrow[:, :], sc2m[:, :])
    S.dma(tableS[:, :], tchunks[:, :])

    # ---- P1: top-200 ----
    m8 = sb("m8", [SPC, 8])
    i200 = sb("i200", [SPC, K0], DT.uint32)
    for r in range(K0 // 8):
        sl = slice(8 * r, 8 * r + 8)
        S.v(lambda e, sl=sl: nc.vector.max(out=m8[:, :], in_=xrow[:, :]))
        S.v(lambda e, sl=sl: nc.vector.max_index(out=i200[:, sl], in_max=m8[:, :], in_values=xrow[:, :]))
        S.v(lambda e, sl=sl: nc.vector.match_replace(out=xrow[:, :], in_to_replace=m8[:, :],
                                                     in_values=xrow[:, :], imm_value=-1e30))
    idxI = sb("idxI", [SPC, K0], DT.int32)
    loI = sb("loI", [SPC, K0], DT.int32)
    hiI = sb("hiI", [SPC, K0], DT.int32)
    loF = sb("loF", [SPC, K0])
    hiF = sb("hiF", [SPC, K0])
    CP(idxI[:, :], i200[:, :])
    TS(loI[:, :], idxI[:, :], 127, None, OP.bitwise_and)
    TS(hiI[:, :], idxI[:, :], 7, None, OP.logical_shift_right)
    CP(loF[:, :], loI[:, :])
    CP(hiF[:, :], hiI[:, :])
    es1.close()

    # ---- P2: gather via PE one-hot matmuls ----
    ident = sb("ident", [128, 128])
    S.g(lambda e: nc.gpsimd.memset(ident[:, :], 0.0))
    S.g(lambda e: nc.gpsimd.affine_select(out=ident[:, :], in_=ident[:, :],
                                          compare_op=OP.not_equal, fill=1.0,
                                          base=0, pattern=[[-1, 128]], channel_multiplier=1))
    io128I = sb("io128I", [SPC, 128], DT.int32)
    io128F = sb("io128F", [SPC, 128])
    io16I = sb("io16I", [SPC, 16], DT.int32)
    io16F = sb("io16F", [SPC, 16])
    posI = sb("posI", [SPC, K0], DT.int32)
    posF = sb("posF", [SPC, K0])
    S.g(lambda e: nc.gpsimd.iota(io128I[:, :], pattern=[[1, 128]], base=0, channel_multiplier=0))
    S.g(lambda e: nc.gpsimd.iota(io16I[:, :], pattern=[[1, 16]], base=0, channel_multiplier=0))
    S.g(lambda e: nc.gpsimd.iota(posI[:, :], pattern=[[1, K0]], base=0, channel_multiplier=0))
    CP(io128F[:, :], io128I[:, :])
    CP(io16F[:, :], io16I[:, :])
    CP(posF[:, :], posI[:, :])

    g6 = sb("g6", [SPC, K0, 6])
    es2 = ExitStack()
    ohq = sbR(es2, "ohq", [SPC, 4, 128])
    ohT = sbR(es2, "ohT", [128, 4, 64])
    cmp16 = sbR(es2, "cmp16", [SPC, 4, 16])
    msel = sbR(es2, "msel", [SPC, 4, 16, 6])
    psT = ctx.enter_context(nc.psum_tensor("psT", [128, 64], DT.float32))
    psS = ctx.enter_context(nc.psum_tensor("psS", [SPC, 4, 96], DT.float32))
    for q in range(K0 // 4):
        r0 = 4 * q
        TT(ohq[:, :, :], io128F[:, :].unsqueeze(1).to_broadcast([SPC, 4, 128]),
           loF[:, r0:r0 + 4].unsqueeze(2).to_broadcast([SPC, 4, 128]), OP.is_equal)
        for i in range(4):
            S.p(lambda e, i=i: nc.tensor.transpose(out=psT[:, :], in_=ohq[:, i, :],
                                                   identity=ident[0:SPC, 0:SPC]))
            CP(ohT[:, i, :], psT[:, :])
            S.p(lambda e, i=i: nc.tensor.matmul(out=psS[:, i, :], lhsT=ohT[:, i, :],
                                                rhs=tableS[:, :], start=True, stop=True))
        TT(cmp16[:, :, :], io16F[:, :].unsqueeze(1).to_broadcast([SPC, 4, 16]),
           hiF[:, r0:r0 + 4].unsqueeze(2).to_broadcast([SPC, 4, 16]), OP.is_equal)
        TT(msel[:, :, :, :], psS[:, :, :].rearrange("p a (c x) -> p a c x", c=16),
           cmp16[:, :, :].unsqueeze(3).to_broadcast([SPC, 4, 16, 6]), OP.mult)
        RD(g6[:, r0:r0 + 4, :], msel[:, :, :, :].transpose([0, 1, 3, 2]))
    gx = sb("gx", [SPC, 3, K0])
    gy = sb("gy", [SPC, 3, K0])
    for c in range(3):
        CP(gx[:, c, :], g6[:, :, c])
        CP(gy[:, c, :], g6[:, :, c + 3])
    es2.close()

    # ---- P3: H bits (bf16 200x200) ----
    H = sb("H", [SPC, K0, K0], DT.bfloat16)
    B = 10
    es3 = ExitStack()
    dxs = sbR(es3, "dxs", [SPC, B, 3, K0])
    d2a = sbR(es3, "d2a", [SPC, B, K0])
    d2b = sbR(es3, "d2b", [SPC, B, K0])
    qq = sbR(es3, "qq", [SPC, B, K0])
    for bi in range(K0 // B):
        a0 = bi * B
        for (gsrc, dst) in ((gx, d2a), (gy, d2b)):
            rows4 = gsrc[:, :, :].unsqueeze(1).to_broadcast([SPC, B, 3, K0])
            cols4 = gsrc[:, :, a0:a0 + B].transpose([0, 2, 1]).unsqueeze(3).to_broadcast([SPC, B, 3, K0])
            TT(dxs[:, :, :, :], rows4, cols4, OP.subtract)
            TT(dxs[:, :, :, :], dxs[:, :, :, :], dxs[:, :, :, :], OP.mult)
            TT(dst[:, :, :], dxs[:, :, 0, :], dxs[:, :, 1, :], OP.add)
            TT(dst[:, :, :], dst[:, :, :], dxs[:, :, 2, :], OP.add)
        TT(qq[:, :, :], d2a[:, :, :], d2b[:, :, :], OP.add)
        TT(d2a[:, :, :], d2a[:, :, :], d2b[:, :, :], OP.subtract)
        TT(d2a[:, :, :], d2a[:, :, :], d2a[:, :, :], OP.mult)
        TS(d2b[:, :, :], qq[:, :, :], TWO_T2, T4, OP.mult, OP.subtract)
        TT(d2a[:, :, :], d2a[:, :, :], d2b[:, :, :], OP.is_lt)
        TS(d2b[:, :, :], qq[:, :, :], T2, None, OP.is_lt)
        TT(H[:, a0:a0 + B, :], d2a[:, :, :], d2b[:, :, :], OP.max)
    es3.close()

    # ---- P4: masked filter stages ----
    es4 = ExitStack()
    TMP = sbR(es4, "TMP", [SPC, K0, K0], DT.bfloat16)
    mM = sb("mM", [SPC, K0])
    rF = sb("rF", [SPC, K0])
    lam = sb("lam", [SPC, K0])
    Hl = sb("Hl", [SPC, K0])
    vv = sb("vv", [SPC, K0])
    sc2v = sb("sc2v", [SPC, K0])
    packed = sb("packed", [SPC, K0])
    pcopy = sb("pcopy", [SPC, K0])
    m8s = sb("m8s", [SPC, 104])
    MS(mM[:, :], 1.0)
    CP(rF[:, :], posF[:, :])
    for kf in (100, 50, 25, 12):
        TS(lam[:, :], rF[:, :], 0.0, None, OP.is_equal)
        TT(TMP[:, :, :], H[:, :, :], lam[:, :].unsqueeze(2).to_broadcast([SPC, K0, K0]), OP.mult)
        RD(Hl[:, :], TMP[:, :, :].transpose([0, 2, 1]))
        TT(vv[:, :], Hl[:, :], mM[:, :], OP.mult)
        TT(TMP[:, :, :], H[:, :, :], vv[:, :].unsqueeze(2).to_broadcast([SPC, K0, K0]), OP.mult)
        RD(sc2v[:, :], TMP[:, :, :].transpose([0, 2, 1]))
        TS(packed[:, :], sc2v[:, :], 256.0, 255.0, OP.mult, OP.add)
        TT(packed[:, :], packed[:, :], rF[:, :], OP.subtract)
        TT(packed[:, :], packed[:, :], mM[:, :], OP.mult)
        CP(pcopy[:, :], packed[:, :])
        for r in range(math.ceil(kf / 8)):
            sl = slice(8 * r, 8 * r + 8)
            S.v(lambda e, sl=sl: nc.vector.max(out=m8s[:, sl], in_=pcopy[:, :]))
            S.v(lambda e, sl=sl: nc.vector.match_replace(out=pcopy[:, :], in_to_replace=m8s[:, sl],
                                                         in_values=pcopy[:, :], imm_value=-1.0))
        TS(mM[:, :], packed[:, :], m8s[:, kf - 1:kf], None, OP.is_ge)
        TT(TMP[:, :, :], packed[:, :].unsqueeze(2).to_broadcast([SPC, K0, K0]),
           packed[:, :].unsqueeze(1).to_broadcast([SPC, K0, K0]), OP.is_gt)
        RD(rF[:, :], TMP[:, :, :].transpose([0, 2, 1]))
    es4.close()

    # ---- P5: final compaction + M12 + power iteration ----
    fx = sb("fx", [SPC, 3, 12])
    fy = sb("fy", [SPC, 3, 12])
    oh1 = sb("oh1", [SPC, K0])
    t200 = sb("t200", [SPC, K0])
    for rho in range(12):
        TS(oh1[:, :], rF[:, :], float(rho), None, OP.is_equal)
        for c in range(3):
            TT(t200[:, :], oh1[:, :], gx[:, c, :], OP.mult)
            RD(fx[:, c, rho:rho + 1], t200[:, :])
            TT(t200[:, :], oh1[:, :], gy[:, c, :], OP.mult)
            RD(fy[:, c, rho:rho + 1], t200[:, :])

    dx12 = sb("dx12", [SPC, 12, 3, 12])
    a2s = sb("a2s", [SPC, 12, 12])
    b2s = sb("b2s", [SPC, 12, 12])
    M12 = sb("M12", [SPC, 12, 12])
    for (gsrc, dst) in ((fx, a2s), (fy, b2s)):
        rows4 = gsrc[:, :, :].unsqueeze(1).to_broadcast([SPC, 12, 3, 12])
        cols4 = gsrc[:, :, :].transpose([0, 2, 1]).unsqueeze(3).to_broadcast([SPC, 12, 3, 12])
        TT(dx12[:, :, :, :], rows4, cols4, OP.subtract)
        TT(dx12[:, :, :, :], dx12[:, :, :, :], dx12[:, :, :, :], OP.mult)
        TT(dst[:, :, :], dx12[:, :, 0, :], dx12[:, :, 1, :], OP.add)
        TT(dst[:, :, :], dst[:, :, :], dx12[:, :, 2, :], OP.add)
    TS(a2s[:, :, :], a2s[:, :, :], 1e-12, None, OP.max)
    TS(b2s[:, :, :], b2s[:, :, :], 1e-12, None, OP.max)
    SQRT(a2s[:, :, :], a2s[:, :, :])
    SQRT(b2s[:, :, :], b2s[:, :, :])
    TT(a2s[:, :, :], a2s[:, :, :], b2s[:, :, :], OP.subtract)
    TT(a2s[:, :, :], a2s[:, :, :], a2s[:, :, :], OP.mult)
    TS(M12[:, :, :], a2s[:, :, :], float(F32(1.0) / F32(T2)), None, OP.mult)
    TS(M12[:, :, :], M12[:, :, :], -1.0, None, OP.mult)
    TS(M12[:, :, :], M12[:, :, :], 1.0, None, OP.add)
    TS(M12[:, :, :], M12[:, :, :], 0.0, None, OP.max)
    S.g(lambda e: nc.gpsimd.affine_select(out=M12[:, :, :], in_=M12[:, :, :],
                                          compare_op=OP.not_equal, fill=0.0,
                                          base=0, pattern=[[-1, 12], [1, 12]],
                                          channel_multiplier=0))
    v12 = sb("v12", [SPC, 12])
    t144 = sb("t144", [SPC, 12, 12])
    sq12 = sb("sq12", [SPC, 12])
    nrm = sb("nrm", [SPC, 1])
    MS(v12[:, :], 1.0)
    for _ in range(10):
        TT(t144[:, :, :], M12[:, :, :], v12[:, :].unsqueeze(1).to_broadcast([SPC, 12, 12]), OP.mult)
        RD(v12[:, :], t144[:, :, :])
        TT(sq12[:, :], v12[:, :], v12[:, :], OP.mult)
        RD(nrm[:, :], sq12[:, :])
        SQRT(nrm[:, :], nrm[:, :])
        TS(nrm[:, :], nrm[:, :], 1e-6, None, OP.add)
        RCP(nrm[:, :], nrm[:, :])
        TS(v12[:, :], v12[:, :], nrm[:, 0:1], None, OP.mult)
    w12 = sb("w12", [SPC, 12])
    RD(nrm[:, :], v12[:, :])
    TS(nrm[:, :], nrm[:, :], 1e-6, None, OP.add)
    RCP(nrm[:, :], nrm[:, :])
    TS(w12[:, :], v12[:, :], nrm[:, 0:1], None, OP.mult)

    # ---- P6: Kabsch ----
    t12a = sb("t12a", [SPC, 12])
    t3a = sb("t3a", [SPC, 3])
    cA = sb("cA", [SPC, 3])
    cB = sb("cB", [SPC, 3])
    ws1 = sb("ws1", [SPC, 1])
    Am = sb("Am", [SPC, 3, 12])
    Bm = sb("Bm", [SPC, 3, 12])
    wAm = sb("wAm", [SPC, 3, 12])
    Hm = sb("Hm", [SPC, 9])
    Km = sb("Km", [SPC, 9])
    RD(ws1[:, :], w12[:, :])
    TS(ws1[:, :], ws1[:, :], 1e-6, None, OP.add)
    RCP(ws1[:, :], ws1[:, :])
    for c in range(3):
        TT(t12a[:, :], fx[:, c, :], w12[:, :], OP.mult)
        RD(cA[:, c:c + 1], t12a[:, :])
        TT(t12a[:, :], fy[:, c, :], w12[:, :], OP.mult)
        RD(cB[:, c:c + 1], t12a[:, :])
    TS(cA[:, :], cA[:, :], ws1[:, 0:1], None, OP.mult)
    TS(cB[:, :], cB[:, :], ws1[:, 0:1], None, OP.mult)
    TT(Am[:, :, :], fx[:, :, :], cA[:, :].unsqueeze(2).to_broadcast([SPC, 3, 12]), OP.subtract)
    TT(Bm[:, :, :], fy[:, :, :], cB[:, :].unsqueeze(2).to_broadcast([SPC, 3, 12]), OP.subtract)
    TT(wAm[:, :, :], Am[:, :, :], w12[:, :].unsqueeze(1).to_broadcast([SPC, 3, 12]), OP.mult)
    for i in range(3):
        for j in range(3):
            TT(t12a[:, :], wAm[:, i, :], Bm[:, j, :], OP.mult)
            RD(Hm[:, 3 * i + j:3 * i + j + 1], t12a[:, :])
    for i in range(3):
        for k in range(3):
            TT(t3a[:, :], Hm[:, 3 * i:3 * i + 3], Hm[:, 3 * k:3 * k + 3], OP.mult)
            RD(Km[:, 3 * i + k:3 * i + k + 1], t3a[:, :])

    s1 = lambda name: sb(name, [SPC, 1])
    eqq = s1("eqq"); ts1 = s1("ts1"); ts2 = s1("ts2")
    p1 = s1("p1"); p2v = s1("p2v"); pv = s1("pv"); rp = s1("rp")
    Bk = sb("Bk", [SPC, 9])
    detB = s1("detB"); rr = s1("rr"); cc = s1("cc"); c2 = s1("c2")
    ff = s1("ff"); fp = s1("fp"); ss = s1("ss"); lam1 = s1("lam1"); lam2 = s1("lam2")

    TT(eqq[:, :], Km[:, 0:1], Km[:, 4:5], OP.add)
    TT(eqq[:, :], eqq[:, :], Km[:, 8:9], OP.add)
    TS(eqq[:, :], eqq[:, :], float(F32(1.0) / F32(3.0)), None, OP.mult)
    CP(Bk[:, :], Km[:, :])
    for d in (0, 4, 8):
        TS(Bk[:, d:d + 1], Bk[:, d:d + 1], eqq[:, 0:1], None, OP.subtract)
    TT(p1[:, :], Km[:, 1:2], Km[:, 1:2], OP.mult)
    TT(ts1[:, :], Km[:, 2:3], Km[:, 2:3], OP.mult)
    TT(p1[:, :], p1[:, :], ts1[:, :], OP.add)
    TT(ts1[:, :], Km[:, 5:6], Km[:, 5:6], OP.mult)
    TT(p1[:, :], p1[:, :], ts1[:, :], OP.add)
    TT(p2v[:, :], Bk[:, 0:1], Bk[:, 0:1], OP.mult)
    TT(ts1[:, :], Bk[:, 4:5], Bk[:, 4:5], OP.mult)
    TT(p2v[:, :], p2v[:, :], ts1[:, :], OP.add)
    TT(ts1[:, :], Bk[:, 8:9], Bk[:, 8:9], OP.mult)
    TT(p2v[:, :], p2v[:, :], ts1[:, :], OP.add)
    TS(ts1[:, :], p1[:, :], 2.0, None, OP.mult)
    TT(p2v[:, :], p2v[:, :], ts1[:, :], OP.add)
    TS(pv[:, :], p2v[:, :], float(F32(1.0) / F32(6.0)), None, OP.mult)
    SQRT(pv[:, :], pv[:, :])
    TS(rp[:, :], pv[:, :], 1e-30, None, OP.max)
    RCP(rp[:, :], rp[:, :])
    TS(Bk[:, :], Bk[:, :], rp[:, 0:1], None, OP.mult)
    TT(ts1[:, :], Bk[:, 4:5], Bk[:, 8:9], OP.mult)
    TT(ts2[:, :], Bk[:, 5:6], Bk[:, 5:6], OP.mult)
    TT(ts1[:, :], ts1[:, :], ts2[:, :], OP.subtract)
    TT(detB[:, :], Bk[:, 0:1], ts1[:, :], OP.mult)
    TT(ts1[:, :], Bk[:, 1:2], Bk[:, 8:9], OP.mult)
    TT(ts2[:, :], Bk[:, 5:6], Bk[:, 2:3], OP.mult)
    TT(ts1[:, :], ts1[:, :], ts2[:, :], OP.subtract)
    TT(ts1[:, :], Bk[:, 1:2], ts1[:, :], OP.mult)
    TT(detB[:, :], detB[:, :], ts1[:, :], OP.subtract)
    TT(ts1[:, :], Bk[:, 1:2], Bk[:, 5:6], OP.mult)
    TT(ts2[:, :], Bk[:, 4:5], Bk[:, 2:3], OP.mult)
    TT(ts1[:, :], ts1[:, :], ts2[:, :], OP.subtract)
    TT(ts1[:, :], Bk[:, 2:3], ts1[:, :], OP.mult)
    TT(detB[:, :], detB[:, :], ts1[:, :], OP.add)
    TS(rr[:, :], detB[:, :], 0.5, None, OP.mult)
    TS(rr[:, :], rr[:, :], -1.0, None, OP.max)
    TS(rr[:, :], rr[:, :], 1.0, None, OP.min)
    MS(cc[:, :], 1.0)
    for _ in range(6):
        TT(c2[:, :], cc[:, :], cc[:, :], OP.mult)
        TT(ff[:, :], c2[:, :], cc[:, :], OP.mult)
        TS(ff[:, :], ff[:, :], 4.0, None, OP.mult)
        TS(ts1[:, :], cc[:, :], 3.0, None, OP.mult)
        TT(ff[:, :], ff[:, :], ts1[:, :], OP.subtract)
        TT(ff[:, :], ff[:, :], rr[:, :], OP.subtract)
        TS(fp[:, :], c2[:, :], 12.0, 3.0, OP.mult, OP.subtract)
        TS(fp[:, :], fp[:, :], 1e-6, None, OP.max)
        RCP(fp[:, :], fp[:, :])
        TT(ff[:, :], ff[:, :], fp[:, :], OP.mult)
        TT(cc[:, :], cc[:, :], ff[:, :], OP.subtract)
        TS(cc[:, :], cc[:, :], 0.5, None, OP.max)
        TS(cc[:, :], cc[:, :], 1.0, None, OP.min)
    TT(c2[:, :], cc[:, :], cc[:, :], OP.mult)
    TS(ss[:, :], c2[:, :], -1.0, 1.0, OP.mult, OP.add)
    TS(ss[:, :], ss[:, :], 0.0, None, OP.max)
    SQRT(ss[:, :], ss[:, :])
    TT(ts1[:, :], pv[:, :], cc[:, :], OP.mult)
    TS(ts1[:, :], ts1[:, :], 2.0, None, OP.mult)
    TT(lam1[:, :], eqq[:, :], ts1[:, :], OP.add)
    TS(ts1[:, :], cc[:, :], -0.5, None, OP.mult)
    TS(ts2[:, :], ss[:, :], float(F32(np.sqrt(3.0) / 2.0)), None, OP.mult)
    TT(ts1[:, :], ts1[:, :], ts2[:, :], OP.add)
    TT(ts1[:, :], pv[:, :], ts1[:, :], OP.mult)
    TS(ts1[:, :], ts1[:, :], 2.0, None, OP.mult)
    TT(lam2[:, :], eqq[:, :], ts1[:, :], OP.add)

    Ae = sb("Ae", [SPC, 9])
    cr1 = sb("cr1", [SPC, 3]); cr2 = sb("cr2", [SPC, 3]); cr3 = sb("cr3", [SPC, 3])
    n1 = s1("n1"); n2 = s1("n2"); n3 = s1("n3")
    aa1 = s1("aa1"); aa2 = s1("aa2"); aa3 = s1("aa3")
    u1 = sb("u1", [SPC, 3]); u2 = sb("u2", [SPC, 3]); u3 = sb("u3", [SPC, 3])

    def cross_rows(out, r0s, r1s):
        for (o, x, y) in ((0, 1, 2), (1, 2, 0), (2, 0, 1)):
            TT(ts1[:, :], r0s[:, x:x + 1], r1s[:, y:y + 1], OP.mult)
            TT(ts2[:, :], r0s[:, y:y + 1], r1s[:, x:x + 1], OP.mult)
            TT(out[:, o:o + 1], ts1[:, :], ts2[:, :], OP.subtract)

    def eigvec(uout, lamv):
        CP(Ae[:, :], Km[:, :])
        for d in (0, 4, 8):
            TS(Ae[:, d:d + 1], Ae[:, d:d + 1], lamv[:, 0:1], None, OP.subtract)
        r0s, r1s, r2s = Ae[:, 0:3], Ae[:, 3:6], Ae[:, 6:9]
        cross_rows(cr1, r0s, r1s)
        cross_rows(cr2, r1s, r2s)
        cross_rows(cr3, r2s, r0s)
        for (nv, crv) in ((n1, cr1), (n2, cr2), (n3, cr3)):
            TT(t3a[:, :], crv[:, :], crv[:, :], OP.mult)
            RD(nv[:, :], t3a[:, :])
        TT(aa1[:, :], n1[:, :], n2[:, :], OP.is_ge)
        TT(ts1[:, :], n1[:, :], n3[:, :], OP.is_ge)
        TT(aa1[:, :], aa1[:, :], ts1[:, :], OP.mult)
        TS(aa2[:, :], aa1[:, :], -1.0, 1.0, OP.mult, OP.add)
        TT(ts1[:, :], n2[:, :], n3[:, :], OP.is_ge)
        TT(aa2[:, :], aa2[:, :], ts1[:, :], OP.mult)
        TS(aa3[:, :], aa1[:, :], -1.0, 1.0, OP.mult, OP.add)
        TT(aa3[:, :], aa3[:, :], aa2[:, :], OP.subtract)
        TS(uout[:, :], cr1[:, :], aa1[:, 0:1], None, OP.mult)
        STT(uout[:, :], cr2[:, :], aa2[:, 0:1], uout[:, :], OP.mult, OP.add)
        STT(uout[:, :], cr3[:, :], aa3[:, 0:1], uout[:, :], OP.mult, OP.add)
        TT(t3a[:, :], uout[:, :], uout[:, :], OP.mult)
        RD(ts1[:, :], t3a[:, :])
        TS(ts1[:, :], ts1[:, :], 1e-38, None, OP.max)
        SQRT(ts1[:, :], ts1[:, :])
        RCP(ts1[:, :], ts1[:, :])
        TS(uout[:, :], uout[:, :], ts1[:, 0:1], None, OP.mult)

    eigvec(u1, lam1)
    eigvec(u2, lam2)
    TT(t3a[:, :], u1[:, :], u2[:, :], OP.mult)
    RD(ts1[:, :], t3a[:, :])
    STT(u2[:, :], u1[:, :], ts1[:, 0:1], u2[:, :], OP.mult, OP.subtract)
    TS(u2[:, :], u2[:, :], -1.0, None, OP.mult)
    TT(t3a[:, :], u2[:, :], u2[:, :], OP.mult)
    RD(ts1[:, :], t3a[:, :])
    TS(ts1[:, :], ts1[:, :], 1e-38, None, OP.max)
    SQRT(ts1[:, :], ts1[:, :])
    RCP(ts1[:, :], ts1[:, :])
    TS(u2[:, :], u2[:, :], ts1[:, 0:1], None, OP.mult)
    cross_rows(u3, u1, u2)
    wv1 = sb("wv1", [SPC, 3]); wv2 = sb("wv2", [SPC, 3])
    for i in range(3):
        TT(t3a[:, :], Hm[:, i::3], u1[:, :], OP.mult)
        RD(wv1[:, i:i + 1], t3a[:, :])
        TT(t3a[:, :], Hm[:, i::3], u2[:, :], OP.mult)
        RD(wv2[:, i:i + 1], t3a[:, :])
    for wv in (wv1, wv2):
        TT(t3a[:, :], wv[:, :], wv[:, :], OP.mult)
        RD(ts1[:, :], t3a[:, :])
        TS(ts1[:, :], ts1[:, :], 1e-38, None, OP.max)
        SQRT(ts1[:, :], ts1[:, :])
        RCP(ts1[:, :], ts1[:, :])
        TS(wv[:, :], wv[:, :], ts1[:, 0:1], None, OP.mult)
    vv3 = sb("vv3", [SPC, 3])
    cross_rows(vv3, wv1, wv2)
    R9 = sb("R9", [SPC, 9])
    for c in range(3):
        TS(R9[:, 3 * c:3 * c + 3], u1[:, :], wv1[:, c:c + 1], None, OP.mult)
        STT(R9[:, 3 * c:3 * c + 3], u2[:, :], wv2[:, c:c + 1], R9[:, 3 * c:3 * c + 3], OP.mult, OP.add)
        STT(R9[:, 3 * c:3 * c + 3], u3[:, :], vv3[:, c:c + 1], R9[:, 3 * c:3 * c + 3], OP.mult, OP.add)
    t3v = sb("t3v", [SPC, 3])
    for c in range(3):
        TT(t3a[:, :], R9[:, 3 * c:3 * c + 3], cA[:, :], OP.mult)
        RD(ts1[:, :], t3a[:, :])
        TT(t3v[:, c:c + 1], cB[:, c:c + 1], ts1[:, :], OP.subtract)

    # ---- P7: fitness ----
    es7 = ExitStack()
    clS = sbR(es7, "clS", [SPC, 3 * NPTS])
    clT = sbR(es7, "clT", [SPC, 3 * NPTS])
    acc = sbR(es7, "acc", [SPC, NPTS])
    dcv = sbR(es7, "dcv", [SPC, 3, NPTS])
    l2s = sbR(es7, "l2s", [SPC, NPTS])
    sqv = sbR(es7, "sqv", [SPC, NPTS])
    S.dma(clS[:, :], cloudS[:, :].to_broadcast([SPC, 3 * NPTS]))
    S.dma(clT[:, :], cloudT[:, :].to_broadcast([SPC, 3 * NPTS]))
    cnt = sb("cnt", [SPC, 1])
    xv = clS[:, :].rearrange("p (c n) -> p c n", c=3)
    yv = clT[:, :].rearrange("p (c n) -> p c n", c=3)
    for c in range(3):
        TS(acc[:, :], xv[:, 0, :], R9[:, 3 * c:3 * c + 1], t3v[:, c:c + 1], OP.mult, OP.add)
        STT(acc[:, :], xv[:, 1, :], R9[:, 3 * c + 1:3 * c + 2], acc[:, :], OP.mult, OP.add)
        STT(acc[:, :], xv[:, 2, :], R9[:, 3 * c + 2:3 * c + 3], acc[:, :], OP.mult, OP.add)
        TT(dcv[:, c, :], acc[:, :], yv[:, c, :], OP.subtract)
    TT(l2s[:, :], dcv[:, 0, :], dcv[:, 0, :], OP.mult)
    TT(sqv[:, :], dcv[:, 1, :], dcv[:, 1, :], OP.mult)
    TT(l2s[:, :], l2s[:, :], sqv[:, :], OP.add)
    TT(sqv[:, :], dcv[:, 2, :], dcv[:, 2, :], OP.mult)
    TT(l2s[:, :], l2s[:, :], sqv[:, :], OP.add)
    TS(sqv[:, :], l2s[:, :], T2, None, OP.is_lt)
    RD(cnt[:, :], sqv[:, :])

    res = sb("res", [SPC, 16])
    MS(res[:, :], 0.0)
    CP(res[:, 0:9], R9[:, :])
    CP(res[:, 9:12], t3v[:, :])
    CP(res[:, 12:13], cnt[:, :])
    S.dma(outT[:, :], res[:, :])
    es7.close()
    S.emit()
    return nc


def _get_prog():
    if "fused" not in _programs:
        _programs["fused"] = _build()
    return _programs["fused"]


def _run(nc, in_maps):
    import time
    from concourse.bass_utils import run_bass_kernel_spmd
    last = None
    for attempt in range(3):
        try:
            t0 = time.time()
            res = run_bass_kernel_spmd(nc, in_maps, core_ids=list(range(NCORES)))
            _launch_wall.append(time.time() - t0)
            return res.results
        except Exception as e:   # transient device errors: retry
            last = e
    raise last


_cache_cfg = [False]


def _enable_jax_cache():
    if _cache_cfg[0]:
        return
    _cache_cfg[0] = True
    try:
        import jax
        jax.config.update("jax_compilation_cache_dir", "/tmp/_jx_pjrt_cache")
        jax.config.update("jax_persistent_cache_min_compile_time_secs", 0)
        jax.config.update("jax_persistent_cache_min_entry_size_bytes", 0)
    except Exception:
        pass


def kernel(SC2_measure, src_keypts, tgt_keypts):
    _enable_jax_cache()
    _launch_wall.clear()
    SC2 = np.ascontiguousarray(SC2_measure[0], dtype=F32)       # [512, 2048]
    src = np.ascontiguousarray(src_keypts[0], dtype=F32)        # [2048, 3]
    tgt = np.ascontiguousarray(tgt_keypts[0], dtype=F32)

    table6 = np.concatenate([src, tgt], axis=1).astype(F32)     # [2048, 6]
    tchunks = np.ascontiguousarray(
        table6.reshape(16, 128, 6).transpose(1, 0, 2).reshape(128, 96))
    cS = np.ascontiguousarray(src.T.reshape(1, 3 * NPTS)).astype(F32)
    cT = np.ascontiguousarray(tgt.T.reshape(1, 3 * NPTS)).astype(F32)
    in_maps = [{
        "sc2m": np.ascontiguousarray(SC2[c * SPC:(c + 1) * SPC]),
        "tchunks": tchunks, "cloudS": cS, "cloudT": cT,
    } for c in range(NCORES)]

    nc = _get_prog()
    for _try in range(3):
        res = _run(nc, in_maps)
        out = np.concatenate([res[c]["outT"] for c in range(NCORES)], axis=0)
        fit = out[:, 12]
        rnorm = (out[:, 0:9] ** 2).sum(axis=1)
        ok = ((fit == np.round(fit)).all() and (fit >= 0).all() and (fit <= NPTS).all()
              and np.isfinite(out).all() and (np.abs(rnorm - 3.0) < 0.5).all())
        if ok:
            break
    best = int(np.argmax(fit))
    T = np.zeros((1, 4, 4), F32)
    T[0, :3, :3] = out[best, 0:9].reshape(3, 3)
    T[0, :3, 3] = out[best, 9:12]
    T[0, 3, 3] = 1.0
    return T
